# revision 17
# baseline (speedup 1.0000x reference)
"""Energy Transformer descent kernel for 8 Trainium2 NeuronCores.

Problem: 12 steps of gradient descent on
  E(x) = -(1/beta) sum logsumexp(beta q k^T) - 0.5 sum relu(g xi^T)^2,
  g = LayerNorm(x; gamma, delta), q = g Wq_h, k = g Wk_h.

Sharding: data-parallel over batch B=4 -> core pairs (2b, 2b+1); within a
pair core j owns TOKENS j*256..(j+1)*256 for the attention-query /
LayerNorm-backward work and MEMORIES j*1536..(j+1)*1536 for the Hopfield
term (full-width 512-token matmuls there).  Three pair-collectives per
step, two of them overlapped with compute:
  RS2: ReduceScatter(dg_hop [D, N] f32) -> own-token columns; issued right
       after the Hopfield phase, lands during projections+attention.
  RS1: ReduceScatter(dk^T [EW, N] bf16) -> own-token columns; issued after
       the head loop.
  AG:  AllGather(dx own-half) -> full dx for the x update (exposed).

The SPMD program is identical on both cores: token/memory ownership enters
only through per-core inputs (a one-hot selection matrix `sel` and the
xi/xit shards) and the rank-major layouts of the collective buffers.

Host-side folding (delta must be zero, which the problem guarantees):
  wq = sqrt(beta) diag(gamma) Wq   (likewise wk)
  wqt = (1/sqrt(beta)) (diag(gamma) Wq)^T   (likewise wkt)
  xi' = xi diag(gamma)
All matmuls run in bf16 (fp32 PSUM accumulation); fp8 was measured to
break the 2e-2 gate.  Softmax normalisation is folded into the P-transpose
by multiplying with diag(1/rowsum) instead of the identity, and into the
dk^T matmul by pre-scaling q rows.  PSUM plan (8 banks): pw 2 + ps2 3 +
pacc 3, where pacc holds the Hopfield-bwd chains (two passes over d-chunk
halves) and is then reused for the attention dgT chains (zero-init via a
dummy matmul; interleaved half-chains must never each use start=True).
"""

import numpy as np

import concourse.bass as bass
import concourse.tile as tile
from concourse import bacc, mybir

STEPS = 12
ALPHA = 0.125
EPS = 1e-5
B, N, D, H, HD, M = 4, 512, 768, 12, 64, 3072
P = 128
NT = N // P   # 4 full-token chunks
OC = 2        # own-token chunks
NL = OC * P   # 256 own tokens
DT = D // P   # 6 embed chunks
EW = H * HD   # 768 head width (all heads)
ET = EW // P  # 6 head-dim chunks
ML = M // 2   # 1536 own memories
MT2 = ML // P  # 12 own memory chunks
F32 = mybir.dt.float32
BF16 = mybir.dt.bfloat16
AF = mybir.ActivationFunctionType
OP = mybir.AluOpType

REPLICA_GROUPS = [[0, 1], [2, 3], [4, 5], [6, 7]]


def build_kernel(steps=STEPS, with_cc=True, debug_dump=False):
    nc = bacc.Bacc("TRN2", target_bir_lowering=False, debug=False, num_devices=8)

    x_in = nc.declare_dram_parameter("x", [N, D], F32, isOutput=False)
    sel_d = nc.declare_dram_parameter("sel", [N, NL], BF16, isOutput=False)
    wq_d = nc.declare_dram_parameter("wq", [D, EW], BF16, isOutput=False)
    wk_d = nc.declare_dram_parameter("wk", [D, EW], BF16, isOutput=False)
    wqt_d = nc.declare_dram_parameter("wqt", [EW, D], BF16, isOutput=False)
    wkt_d = nc.declare_dram_parameter("wkt", [EW, D], BF16, isOutput=False)
    xi_d = nc.declare_dram_parameter("xi", [ML, D], BF16, isOutput=False)
    xit_d = nc.declare_dram_parameter("xit", [D, ML], BF16, isOutput=False)
    x_out = nc.declare_dram_parameter("x_out", [N, D], F32, isOutput=True)
    dbg = {}
    if debug_dump:
        for nm, shp, dt_ in (("xhat", [N, D], BF16), ("xh_own", [NL, D], F32),
                             ("gT", [D, N], BF16), ("hopT_own", [D, NL], BF16),
                             ("q_own", [NL, EW], BF16), ("kT", [EW, N], BF16),
                             ("U0", [NL, N], BF16), ("PT0", [N, NL], BF16),
                             ("dqTst", [EW, NL], BF16), ("dkTst", [EW, N], BF16),
                             ("dkT_own", [EW, NL], BF16), ("dgTs", [D, NL], BF16),
                             ("dg_own", [NL, D], F32), ("rstd_own", [NL, 1], F32),
                             ("s01", [NL, 2], F32), ("dxb", [NL, D], BF16)):
            dbg[nm] = nc.declare_dram_parameter("o_" + nm, shp, dt_, isOutput=True)

    def dump(nm, ap, pdim=P):
        if debug_dump:
            nc.sync.dma_start(out=dbg[nm].rearrange("(a p) b -> p a b", p=pdim), in_=ap)

    with tile.TileContext(nc) as tc:
        import contextlib

        with contextlib.ExitStack() as ctx:
            consts = ctx.enter_context(tc.tile_pool(name="consts", bufs=1))
            work = ctx.enter_context(tc.tile_pool(name="work", bufs=1))
            upool = ctx.enter_context(tc.tile_pool(name="upool", bufs=2))
            ptool = ctx.enter_context(tc.tile_pool(name="ptool", bufs=2))
            rtp = ctx.enter_context(tc.tile_pool(name="rtp", bufs=1))
            stats = ctx.enter_context(tc.tile_pool(name="stats", bufs=4))
            # PSUM: pw 2 + ps2 3 + pacc 3 = 8 banks
            pw = ctx.enter_context(tc.tile_pool(name="pw", bufs=2, space="PSUM"))
            ps2 = ctx.enter_context(tc.tile_pool(name="ps2", bufs=3, space="PSUM"))
            pacc = ctx.enter_context(tc.tile_pool(name="pacc", bufs=1, space="PSUM"))
            drp = ctx.enter_context(tc.tile_pool(name="drp", bufs=2, space="DRAM"))

            # ---- resident tensors ----
            wq_sb = consts.tile([P, DT, EW], BF16)
            nc.sync.dma_start(out=wq_sb[:], in_=wq_d.rearrange("(dt p) e -> p dt e", p=P))
            wk_sb = consts.tile([P, DT, EW], BF16)
            nc.sync.dma_start(out=wk_sb[:], in_=wk_d.rearrange("(dt p) e -> p dt e", p=P))
            wqt_sb = consts.tile([P, ET, D], BF16)
            nc.sync.dma_start(out=wqt_sb[:], in_=wqt_d.rearrange("(et p) d -> p et d", p=P))
            wkt_sb = consts.tile([P, ET, D], BF16)
            nc.sync.dma_start(out=wkt_sb[:], in_=wkt_d.rearrange("(et p) d -> p et d", p=P))
            xi_sb = consts.tile([P, MT2, D], BF16)
            nc.sync.dma_start(out=xi_sb[:], in_=xi_d.rearrange("(mt p) d -> p mt d", p=P))
            xit_sb = consts.tile([P, DT, ML], BF16)
            nc.sync.dma_start(out=xit_sb[:], in_=xit_d.rearrange("(dt p) m -> p dt m", p=P))
            sel_sb = consts.tile([P, NT, NL], BF16)
            nc.sync.dma_start(out=sel_sb[:], in_=sel_d.rearrange("(nt p) c -> p nt c", p=P))
            sel32 = consts.tile([P, NT, NL], F32)
            nc.vector.tensor_copy(out=sel32[:], in_=sel_sb[:])
            x_sb = consts.tile([P, NT, D], F32)
            nc.sync.dma_start(out=x_sb[:], in_=x_in.rearrange("(nt p) d -> p nt d", p=P))

            from concourse.masks import make_identity

            ident_f = consts.tile([P, P], F32)
            make_identity(nc, ident_f[:])
            ident_b = consts.tile([P, P], BF16)
            nc.vector.tensor_copy(out=ident_b[:], in_=ident_f[:])
            eps_t = consts.tile([P, 1], F32)
            nc.vector.memset(eps_t[:], EPS)
            zl_t = consts.tile([1, P], BF16)
            nc.vector.memset(zl_t[:], 0.0)
            zr_t = consts.tile([1, N], BF16)
            nc.vector.memset(zr_t[:], 0.0)

            for step in range(steps):
                # ======== LayerNorm forward (full tokens) ========
                xhat = work.tile([P, NT, D], BF16, tag="xhat")
                rr_t = stats.tile([P, NT], F32, tag="rr")
                for nt in range(NT):
                    xt = x_sb[:, nt, :]
                    st = stats.tile([P, 3, 6], F32, tag="bnst")
                    xg = xt.rearrange("p (g s) -> p g s", s=256)
                    for gs in range(3):
                        nc.vector.bn_stats(out=st[:, gs, :], in_=xg[:, gs, :])
                    mv = stats.tile([P, 2], F32, tag="mv")
                    nc.vector.bn_aggr(out=mv[:], in_=st[:])
                    rrx = rr_t[:, nt : nt + 1]
                    nc.scalar.activation(out=rrx, in_=mv[:, 1:2], func=AF.Sqrt, bias=eps_t[:], scale=1.0)
                    nc.vector.reciprocal(out=rrx, in_=rrx)
                    nmu = stats.tile([P, 1], F32, tag="nmu")
                    nc.vector.scalar_tensor_tensor(
                        out=nmu[:], in0=mv[:, 0:1], scalar=-1.0, in1=rrx, op0=OP.mult, op1=OP.mult,
                    )
                    nc.vector.tensor_scalar(
                        out=xhat[:, nt, :], in0=xt, scalar1=rrx, scalar2=nmu[:],
                        op0=OP.mult, op1=OP.add,
                    )

                # ======== gT = xhat^T (full) ========
                gT = work.tile([P, DT, N], BF16, tag="gT")
                for dt in range(DT):
                    pg = ps2.tile([P, N], BF16, tag="ps2")
                    for nt in range(NT):
                        nc.tensor.transpose(pg[:, nt * P : (nt + 1) * P], xhat[:, nt, dt * P : (dt + 1) * P], ident_b[:])
                    nc.vector.tensor_copy(out=gT[:, dt, :], in_=pg[:])
                if debug_dump and step == 0:
                    dump("xhat", xhat[:])
                    dump("gT", gT[:])

                # ======== own-token selection (via sel matmuls) ========
                xhat_own = work.tile([P, OC, D], F32, tag="xh_own")
                xhat_own_b = work.tile([P, OC, D], BF16, tag="ocd_b")
                for oc in range(OC):
                    pa = pw.tile([P, 512], F32, tag="pw")
                    pb = ps2.tile([P, 512], F32, tag="ps2")
                    for nt in range(NT):
                        lh = sel_sb[:, nt, oc * P : (oc + 1) * P]
                        nc.tensor.matmul(pa[:], lh, xhat[:, nt, 0:512], start=(nt == 0), stop=(nt == NT - 1))
                        nc.tensor.matmul(pb[:, :256], lh, xhat[:, nt, 512:768], start=(nt == 0), stop=(nt == NT - 1))
                    nc.vector.tensor_copy(out=xhat_own[:, oc, 0:512], in_=pa[:])
                    nc.vector.tensor_copy(out=xhat_own[:, oc, 512:768], in_=pb[:, :256])
                    nc.vector.tensor_copy(out=xhat_own_b[:, oc, 0:512], in_=pa[:])
                    nc.vector.tensor_copy(out=xhat_own_b[:, oc, 512:768], in_=pb[:, :256])
                rstd_own = stats.tile([P, OC], F32, tag="rstd_own")
                for oc in range(OC):
                    pr = ps2.tile([P, 1], F32, tag="ps2")
                    for nt in range(NT):
                        nc.tensor.matmul(
                            pr[:], sel32[:, nt, oc * P : (oc + 1) * P], rr_t[:, nt : nt + 1],
                            start=(nt == 0), stop=(nt == NT - 1),
                        )
                    nc.vector.tensor_copy(out=rstd_own[:, oc : oc + 1], in_=pr[:])
                gT_own = work.tile([P, DT, NL], BF16, tag="gT_own")
                for dt in range(DT):
                    pg = ps2.tile([P, NL], BF16, tag="ps2")
                    for oc in range(OC):
                        nc.tensor.transpose(pg[:, oc * P : (oc + 1) * P], xhat_own_b[:, oc, dt * P : (dt + 1) * P], ident_b[:])
                    nc.vector.tensor_copy(out=gT_own[:, dt, :], in_=pg[:])
                if debug_dump and step == 0:
                    dump("xh_own", xhat_own[:])
                    nc.sync.dma_start(
                        out=dbg["rstd_own"].rearrange("(c p) o -> p c o", p=P),
                        in_=rstd_own[:].rearrange("p (c o) -> p c o", o=1),
                    )

                # ======== Hopfield (own memories, all tokens) ========
                RTs = rtp.tile([P, MT2, N], BF16, tag="RT")
                for mt in range(MT2):
                    hp = ps2.tile([P, N], F32, tag="ps2")
                    for dt in range(DT):
                        nc.tensor.matmul(
                            hp[:], xit_sb[:, dt, mt * P : (mt + 1) * P], gT[:, dt, :],
                            start=(dt == 0), stop=(dt == DT - 1),
                        )
                    nc.vector.tensor_scalar(
                        out=RTs[:, mt, :], in0=hp[:], scalar1=0.0, scalar2=None, op0=OP.max,
                    )
                # bwd in two passes over d-chunk halves (3 pacc banks each)
                dgh_s = work.tile([P, DT, N], BF16, tag="dgh_s")
                hop_dr = drp.tile([2, D, NL], BF16, tag="rs2_in")
                for dh in range(2):
                    hb = [pacc.tile([P, N], F32, tag=f"c{b}", name=f"hop{dh}{b}") for b in range(3)]
                    for mt in range(MT2):
                        for b in range(3):
                            dt = dh * 3 + b
                            nc.tensor.matmul(
                                hb[b][:], xi_sb[:, mt, dt * P : (dt + 1) * P], RTs[:, mt, :],
                                start=(mt == 0), stop=(mt == MT2 - 1),
                            )
                    for b in range(3):
                        nc.vector.tensor_copy(out=dgh_s[:, dh * 3 + b, :], in_=hb[b][:])
                for r in range(2):
                    nc.sync.dma_start(
                        out=hop_dr[r].rearrange("(dt p) n -> p dt n", p=P),
                        in_=dgh_s[:, :, r * NL : (r + 1) * NL],
                    )
                hopT_own = work.tile([P, DT, NL], BF16, tag="hopT_own")
                if with_cc:
                    rs2_out = drp.tile([D, NL], BF16, tag="rs2_out")
                    nc.gpsimd.collective_compute(
                        "ReduceScatter", OP.add, replica_groups=REPLICA_GROUPS,
                        ins=[hop_dr.opt()], outs=[rs2_out.opt()],
                    )
                    nc.sync.dma_start(out=hopT_own[:], in_=rs2_out.rearrange("(dt p) n -> p dt n", p=P))
                else:
                    nc.sync.dma_start(out=hopT_own[:], in_=hop_dr[0].rearrange("(dt p) n -> p dt n", p=P))
                if debug_dump and step == 0:
                    dump("hopT_own", hopT_own[:])

                # ======== projections ========
                q_own = work.tile([P, OC, EW], BF16, tag="q_own")
                for oc in range(OC):
                    pa = pw.tile([P, 512], F32, tag="pw")
                    pb = ps2.tile([P, 512], F32, tag="ps2")
                    for dt in range(DT):
                        lh = gT_own[:, dt, oc * P : (oc + 1) * P]
                        nc.tensor.matmul(pa[:, :384], lh, wq_sb[:, dt, 0:384], start=(dt == 0), stop=(dt == DT - 1))
                        nc.tensor.matmul(pb[:, :384], lh, wq_sb[:, dt, 384:768], start=(dt == 0), stop=(dt == DT - 1))
                    nc.vector.tensor_copy(out=q_own[:, oc, 0:384], in_=pa[:, :384])
                    nc.vector.tensor_copy(out=q_own[:, oc, 384:768], in_=pb[:, :384])
                k_sb = work.tile([P, NT, EW], BF16, tag="k")
                for nt in range(NT):
                    pa = pw.tile([P, 512], F32, tag="pw")
                    pb = ps2.tile([P, 512], F32, tag="ps2")
                    for dt in range(DT):
                        lh = gT[:, dt, nt * P : (nt + 1) * P]
                        nc.tensor.matmul(pa[:, :384], lh, wk_sb[:, dt, 0:384], start=(dt == 0), stop=(dt == DT - 1))
                        nc.tensor.matmul(pb[:, :384], lh, wk_sb[:, dt, 384:768], start=(dt == 0), stop=(dt == DT - 1))
                    nc.vector.tensor_copy(out=k_sb[:, nt, 0:384], in_=pa[:, :384])
                    nc.vector.tensor_copy(out=k_sb[:, nt, 384:768], in_=pb[:, :384])
                qT = work.tile([P, ET, NL], BF16, tag="qT")
                for et in range(ET):
                    pg = ps2.tile([P, NL], BF16, tag="ps2")
                    for oc in range(OC):
                        nc.tensor.transpose(pg[:, oc * P : (oc + 1) * P], q_own[:, oc, et * P : (et + 1) * P], ident_b[:])
                    nc.vector.tensor_copy(out=qT[:, et, :], in_=pg[:])
                kT = work.tile([P, ET, N], BF16, tag="kT")
                for et in range(ET):
                    pg = ps2.tile([P, N], BF16, tag="ps2")
                    for nt in range(NT):
                        nc.tensor.transpose(pg[:, nt * P : (nt + 1) * P], k_sb[:, nt, et * P : (et + 1) * P], ident_b[:])
                    nc.vector.tensor_copy(out=kT[:, et, :], in_=pg[:])
                if debug_dump and step == 0:
                    dump("q_own", q_own[:])
                    dump("kT", kT[:])

                # ======== attention heads ========
                dqTst = work.tile([P, ET, NL], BF16, tag="dqTst")
                dkTst = work.tile([P, ET, N], BF16, tag="dkTst")
                pq = pk = None
                for h in range(H):
                    et, eo = h // 2, (h % 2) * HD
                    Un = upool.tile([P, OC, N], BF16, tag="Un")
                    sm = stats.tile([P, OC], F32, tag="sm")
                    for oc in range(OC):
                        sc = ps2.tile([P, 512], F32, tag="ps2")
                        nc.tensor.matmul(
                            sc[:], qT[eo : eo + HD, et, oc * P : (oc + 1) * P],
                            kT[eo : eo + HD, et, :], start=True, stop=True,
                        )
                        nc.scalar.activation(out=Un[:, oc, :], in_=sc[:], func=AF.Exp, bias=0.0, scale=1.0)
                        nc.vector.tensor_reduce(
                            out=sm[:, oc : oc + 1], in_=Un[:, oc, :],
                            axis=mybir.AxisListType.X, op=OP.add,
                        )
                    nc.vector.reciprocal(out=sm[:], in_=sm[:])
                    dg_m = stats.tile([P, OC, P], BF16, tag="diag")
                    for oc in range(OC):
                        nc.vector.tensor_scalar_mul(out=dg_m[:, oc, :], in0=ident_b[:], scalar1=sm[:, oc : oc + 1])
                        nc.vector.tensor_scalar_mul(
                            out=q_own[:, oc, h * HD : (h + 1) * HD],
                            in0=q_own[:, oc, h * HD : (h + 1) * HD], scalar1=sm[:, oc : oc + 1],
                        )
                    if debug_dump and step == 0 and h == 0:
                        dump("U0", Un[:])
                    PTn = ptool.tile([P, NT, NL], BF16, tag="PTn")
                    for mt in range(NT):
                        pp = ps2.tile([P, NL], F32, tag="ps2")
                        for oc in range(OC):
                            nc.tensor.matmul(
                                pp[:, oc * P : (oc + 1) * P], Un[:, oc, mt * P : (mt + 1) * P],
                                dg_m[:, oc, :], start=True, stop=True,
                            )
                        nc.vector.tensor_copy(out=PTn[:, mt, :], in_=pp[:])
                    if debug_dump and step == 0 and h == 0:
                        dump("PT0", PTn[:])
                    if h % 2 == 0:
                        pq = pw.tile([P, NL], F32, tag="pw")
                        pk = pw.tile([P, N], F32, tag="pw")
                    for mt in range(NT):
                        nc.tensor.matmul(
                            pq[eo : eo + HD, :], k_sb[:, mt, h * HD : (h + 1) * HD], PTn[:, mt, :],
                            start=(mt == 0), stop=(mt == NT - 1),
                        )
                    for oc in range(OC):
                        nc.tensor.matmul(
                            pk[eo : eo + HD, :], q_own[:, oc, h * HD : (h + 1) * HD], Un[:, oc, :],
                            start=(oc == 0), stop=(oc == OC - 1),
                        )
                    if h % 2 == 1:
                        nc.vector.tensor_copy(out=dqTst[:, et, :], in_=pq[:])
                        nc.vector.tensor_copy(out=dkTst[:, et, :], in_=pk[:])

                if debug_dump and step == 0:
                    dump("dqTst", dqTst[:])
                    dump("dkTst", dkTst[:])

                # ======== pair ReduceScatter of dk^T ========
                dkT_own = work.tile([P, ET, NL], BF16, tag="dkT_own")
                if with_cc:
                    rs_in = drp.tile([2, EW, NL], BF16, tag="rs_in")
                    rs_out = drp.tile([EW, NL], BF16, tag="rs_out")
                    for r in range(2):
                        nc.sync.dma_start(
                            out=rs_in[r].rearrange("(et p) n -> p et n", p=P),
                            in_=dkTst[:, :, r * NL : (r + 1) * NL],
                        )
                    nc.gpsimd.collective_compute(
                        "ReduceScatter", OP.add, replica_groups=REPLICA_GROUPS,
                        ins=[rs_in.opt()], outs=[rs_out.opt()],
                    )
                    nc.sync.dma_start(out=dkT_own[:], in_=rs_out.rearrange("(et p) n -> p et n", p=P))
                else:
                    nc.vector.tensor_copy(out=dkT_own[:], in_=dkTst[:, :, 0:NL])
                if debug_dump and step == 0:
                    dump("dkT_own", dkT_own[:])

                # ======== attention dgT accumulation (3 pacc banks) ========
                dgTb = [pacc.tile([P, N], F32, tag=f"c{b}", name=f"dga{b}") for b in range(3)]
                for b in range(3):
                    nc.tensor.matmul(dgTb[b][:], zl_t[:], zr_t[:], start=True, stop=False)
                for dt in range(DT):
                    b, half = dt // 2, dt % 2
                    for et in range(ET):
                        nc.tensor.matmul(
                            dgTb[b][:, half * NL : (half + 1) * NL],
                            wqt_sb[:, et, dt * P : (dt + 1) * P], dqTst[:, et, :],
                            start=False, stop=False,
                        )
                for dt in range(DT):
                    b, half = dt // 2, dt % 2
                    for et in range(ET):
                        nc.tensor.matmul(
                            dgTb[b][:, half * NL : (half + 1) * NL],
                            wkt_sb[:, et, dt * P : (dt + 1) * P], dkT_own[:, et, :],
                            start=False, stop=(et == ET - 1 and half == 1),
                        )

                # ======== combine + transpose + LayerNorm backward ========
                dgTs = work.tile([P, DT, NL], BF16, tag="qT")
                for b in range(3):
                    nc.vector.tensor_tensor(
                        out=dgTs[:, 2 * b : 2 * b + 2, :].rearrange("p t n -> p (t n)"),
                        in0=dgTb[b][:],
                        in1=hopT_own[:, 2 * b : 2 * b + 2, :].rearrange("p t n -> p (t n)"),
                        op=OP.add,
                    )
                if debug_dump and step == 0:
                    dump("dgTs", dgTs[:])
                dg_own = work.tile([P, OC, D], F32, tag="dg_own")
                dxb = work.tile([P, OC, D], BF16, tag="ocd_b")
                m1s = stats.tile([P, OC], F32, tag="m1s")
                u2s = stats.tile([P, OC], F32, tag="u2s")
                for oc in range(OC):
                    pg = ps2.tile([P, D], BF16, tag="ps2")
                    for dt in range(DT):
                        nc.tensor.transpose(pg[:, dt * P : (dt + 1) * P], dgTs[:, dt, oc * P : (oc + 1) * P], ident_b[:])
                    nc.vector.scalar_tensor_tensor(
                        out=dg_own[:, oc, :], in0=pg[:], scalar=rstd_own[:, oc : oc + 1],
                        in1=xhat_own[:, oc, :], op0=OP.mult, op1=OP.bypass,
                        accum_out=m1s[:, oc : oc + 1],
                    )
                    prod = work.tile([P, D], F32, tag="prod")
                    nc.vector.scalar_tensor_tensor(
                        out=prod[:], in0=dg_own[:, oc, :], scalar=1.0, in1=xhat_own[:, oc, :],
                        op0=OP.mult, op1=OP.mult, accum_out=u2s[:, oc : oc + 1],
                    )
                s01 = stats.tile([P, OC, 2], F32, tag="s01")
                nc.vector.tensor_scalar(
                    out=s01[:, :, 0], in0=u2s[:], scalar1=1.0 / D, scalar2=None, op0=OP.mult,
                )
                nc.vector.tensor_scalar(
                    out=s01[:, :, 1], in0=m1s[:], scalar1=1.0 / D, scalar2=None, op0=OP.mult,
                )
                for oc in range(OC):
                    nc.vector.ln_bwd_dx(
                        out=dxb[:, oc, :], dy=dg_own[:, oc, :], x_hat=xhat_own[:, oc, :],
                        mean_dyx=s01[:, oc, 0:1], mean_dy=s01[:, oc, 1:2], scale=ALPHA,
                    )
                if debug_dump and step == 0:
                    dump("dg_own", dg_own[:])
                    dump("s01", s01[:])
                    dump("dxb", dxb[:])

                # ======== pair AllGather of dx; update x ========
                dxg = work.tile([P, NT, D], BF16, tag="k")
                if with_cc:
                    ag_in = drp.tile([NL, D], BF16, tag="ag_in")
                    ag_out = drp.tile([N, D], BF16, tag="ag_out")
                    nc.sync.dma_start(out=ag_in.rearrange("(oc p) d -> p oc d", p=P), in_=dxb[:])
                    nc.gpsimd.collective_compute(
                        "AllGather", OP.bypass, replica_groups=REPLICA_GROUPS,
                        ins=[ag_in.opt()], outs=[ag_out.opt()],
                    )
                    nc.sync.dma_start(out=dxg[:], in_=ag_out.rearrange("(nt p) d -> p nt d", p=P))
                else:
                    nc.vector.memset(dxg[:], 0.0)
                    nc.vector.tensor_copy(
                        out=dxg[:, 0:OC, :].rearrange("p t d -> p (t d)"),
                        in_=dxb[:].rearrange("p t d -> p (t d)"),
                    )
                for nt in range(NT):
                    nc.vector.scalar_tensor_tensor(
                        out=x_sb[:, nt, :], in0=dxg[:, nt, :], scalar=1.0, in1=x_sb[:, nt, :],
                        op0=OP.mult, op1=OP.add,
                    )

            for nt in range(NT):
                nc.sync.dma_start(out=x_out[nt * P : (nt + 1) * P, :], in_=x_sb[:, nt, :])

    nc.compile()
    return nc


def _prep_inputs(x, gamma, delta, Wq, Wk, xi):
    """Build the 8 per-core input dicts (host-side sharding + weight folding)."""
    assert np.allclose(delta, 0.0), "kernel requires delta == 0"
    import ml_dtypes

    bf = ml_dtypes.bfloat16
    beta_sqrt = np.float32(1.0 / np.sqrt(np.sqrt(np.float32(HD))))
    g = gamma.astype(np.float32)
    wq = ((Wq * g[None, :, None]).transpose(1, 0, 2).reshape(D, EW) * beta_sqrt).astype(bf)
    wk = ((Wk * g[None, :, None]).transpose(1, 0, 2).reshape(D, EW) * beta_sqrt).astype(bf)
    wqt = ((Wq * g[None, :, None]).transpose(0, 2, 1).reshape(EW, D) / beta_sqrt).astype(bf)
    wkt = ((Wk * g[None, :, None]).transpose(0, 2, 1).reshape(EW, D) / beta_sqrt).astype(bf)
    xi_f = (xi * g[None, :]).astype(np.float32)
    sels, xis, xits = [], [], []
    for j in range(2):
        s = np.zeros((N, NL), dtype=bf)
        s[np.arange(j * NL, (j + 1) * NL), np.arange(NL)] = 1
        sels.append(s)
        sh = xi_f[j * ML : (j + 1) * ML]
        xis.append(np.ascontiguousarray(sh).astype(bf))
        xits.append(np.ascontiguousarray(sh.T).astype(bf))
    in_maps = []
    for c in range(8):
        b, j = c // 2, c % 2
        in_maps.append(
            {
                "x": np.ascontiguousarray(x[b]),
                "sel": sels[j],
                "wq": wq, "wk": wk, "wqt": wqt, "wkt": wkt,
                "xi": xis[j], "xit": xits[j],
            }
        )
    return in_maps


_NC_CACHE = {}


def _get_nc(steps=STEPS, with_cc=True):
    key = (steps, with_cc)
    if key not in _NC_CACHE:
        _NC_CACHE[key] = build_kernel(steps, with_cc)
    return _NC_CACHE[key]


def kernel(x, gamma, delta, Wq, Wk, xi):
    from concourse.bass_utils import run_bass_kernel_spmd

    x = np.asarray(x, dtype=np.float32)
    in_maps = _prep_inputs(
        x,
        np.asarray(gamma, np.float32),
        np.asarray(delta, np.float32),
        np.asarray(Wq, np.float32),
        np.asarray(Wk, np.float32),
        np.asarray(xi, np.float32),
    )
    nc = _get_nc()
    res = run_bass_kernel_spmd(nc, in_maps, list(range(8)))
    out = np.stack([res.results[2 * b]["x_out"] for b in range(B)], axis=0)
    return out.astype(np.float32)


# revision 19
# speedup vs baseline: 1.1609x; 1.1609x over previous
"""Energy Transformer descent kernel for 8 Trainium2 NeuronCores.

Problem: 12 steps of gradient descent on
  E(x) = -(1/beta) sum logsumexp(beta q k^T) - 0.5 sum relu(g xi^T)^2,
  g = LayerNorm(x; gamma, delta), q = g Wq_h, k = g Wk_h.

Sharding: data-parallel over batch B=4 -> core pairs (2b, 2b+1); within a
pair core j owns TOKENS j*256..(j+1)*256 for the attention-query /
LayerNorm-backward work and MEMORIES j*1536..(j+1)*1536 for the Hopfield
term (full-width 512-token matmuls there).  Three pair-collectives per
step, two of them overlapped with compute:
  RS2: ReduceScatter(dg_hop [D, N] f32) -> own-token columns; issued right
       after the Hopfield phase, lands during projections+attention.
  RS1: ReduceScatter(dk^T [EW, N] bf16) -> own-token columns; issued after
       the head loop.
  AG:  AllGather(dx own-half) -> full dx for the x update (exposed).

The SPMD program is identical on both cores: token/memory ownership enters
only through per-core inputs (a one-hot selection matrix `sel` and the
xi/xit shards) and the rank-major layouts of the collective buffers.

Host-side folding (delta must be zero, which the problem guarantees):
  wq = sqrt(beta) diag(gamma) Wq   (likewise wk)
  wqt = (1/sqrt(beta)) (diag(gamma) Wq)^T   (likewise wkt)
  xi' = xi diag(gamma)
All matmuls run in bf16 (fp32 PSUM accumulation); fp8 was measured to
break the 2e-2 gate.  Softmax normalisation is folded into the P-transpose
by multiplying with diag(1/rowsum) instead of the identity, and into the
dk^T matmul by pre-scaling q rows.  PSUM plan (8 banks): pw 2 + ps2 3 +
pacc 3, where pacc holds the Hopfield-bwd chains (two passes over d-chunk
halves) and is then reused for the attention dgT chains (zero-init via a
dummy matmul; interleaved half-chains must never each use start=True).
"""

import numpy as np

import concourse.bass as bass
import concourse.tile as tile
from concourse import bacc, mybir

STEPS = 12
ALPHA = 0.125
EPS = 1e-5
B, N, D, H, HD, M = 4, 512, 768, 12, 64, 3072
P = 128
NT = N // P   # 4 full-token chunks
OC = 2        # own-token chunks
NL = OC * P   # 256 own tokens
DT = D // P   # 6 embed chunks
EW = H * HD   # 768 head width (all heads)
ET = EW // P  # 6 head-dim chunks
ML = M // 2   # 1536 own memories
MT2 = ML // P  # 12 own memory chunks
F32 = mybir.dt.float32
BF16 = mybir.dt.bfloat16
AF = mybir.ActivationFunctionType
OP = mybir.AluOpType

REPLICA_GROUPS = [[0, 1], [2, 3], [4, 5], [6, 7]]


def build_kernel(steps=STEPS, with_cc=True, debug_dump=False):
    nc = bacc.Bacc("TRN2", target_bir_lowering=False, debug=False, num_devices=8)

    x_in = nc.declare_dram_parameter("x", [N, D], F32, isOutput=False)
    sel_d = nc.declare_dram_parameter("sel", [N, NL], BF16, isOutput=False)
    wq_d = nc.declare_dram_parameter("wq", [D, EW], BF16, isOutput=False)
    wk_d = nc.declare_dram_parameter("wk", [D, EW], BF16, isOutput=False)
    wqt_d = nc.declare_dram_parameter("wqt", [EW, D], BF16, isOutput=False)
    wkt_d = nc.declare_dram_parameter("wkt", [EW, D], BF16, isOutput=False)
    xi_d = nc.declare_dram_parameter("xi", [ML, D], BF16, isOutput=False)
    xit_d = nc.declare_dram_parameter("xit", [D, ML], BF16, isOutput=False)
    x_out = nc.declare_dram_parameter("x_out", [N, D], F32, isOutput=True)
    dbg = {}
    if debug_dump:
        for nm, shp, dt_ in (("xhat", [N, D], BF16), ("xh_own", [NL, D], F32),
                             ("gT", [D, N], BF16), ("hopT_own", [D, NL], BF16),
                             ("q_own", [NL, EW], BF16), ("kT", [EW, N], BF16),
                             ("U0", [NL, N], BF16), ("PT0", [N, NL], BF16),
                             ("dqTst", [EW, NL], BF16), ("dkTst", [EW, N], BF16),
                             ("dkT_own", [EW, NL], BF16), ("dgTs", [D, NL], BF16),
                             ("dg_own", [NL, D], F32), ("rstd_own", [NL, 1], F32),
                             ("s01", [NL, 2], F32), ("dxb", [NL, D], BF16)):
            dbg[nm] = nc.declare_dram_parameter("o_" + nm, shp, dt_, isOutput=True)

    def dump(nm, ap, pdim=P):
        if debug_dump:
            nc.sync.dma_start(out=dbg[nm].rearrange("(a p) b -> p a b", p=pdim), in_=ap)

    with tile.TileContext(nc) as tc:
        import contextlib

        with contextlib.ExitStack() as ctx:
            consts = ctx.enter_context(tc.tile_pool(name="consts", bufs=1))
            work = ctx.enter_context(tc.tile_pool(name="work", bufs=1))
            upool = ctx.enter_context(tc.tile_pool(name="upool", bufs=2))
            ptool = ctx.enter_context(tc.tile_pool(name="ptool", bufs=2))
            rtp = ctx.enter_context(tc.tile_pool(name="rtp", bufs=1))
            stats = ctx.enter_context(tc.tile_pool(name="stats", bufs=4))
            # PSUM: pw 2 + ps2 3 + pacc 3 = 8 banks
            pw = ctx.enter_context(tc.tile_pool(name="pw", bufs=2, space="PSUM"))
            ps2 = ctx.enter_context(tc.tile_pool(name="ps2", bufs=3, space="PSUM"))
            pacc = ctx.enter_context(tc.tile_pool(name="pacc", bufs=1, space="PSUM"))
            drp = ctx.enter_context(tc.tile_pool(name="drp", bufs=2, space="DRAM"))

            # ---- resident tensors ----
            wq_sb = consts.tile([P, DT, EW], BF16)
            nc.sync.dma_start(out=wq_sb[:], in_=wq_d.rearrange("(dt p) e -> p dt e", p=P))
            wk_sb = consts.tile([P, DT, EW], BF16)
            nc.sync.dma_start(out=wk_sb[:], in_=wk_d.rearrange("(dt p) e -> p dt e", p=P))
            wqt_sb = consts.tile([P, ET, D], BF16)
            nc.sync.dma_start(out=wqt_sb[:], in_=wqt_d.rearrange("(et p) d -> p et d", p=P))
            wkt_sb = consts.tile([P, ET, D], BF16)
            nc.sync.dma_start(out=wkt_sb[:], in_=wkt_d.rearrange("(et p) d -> p et d", p=P))
            xi_sb = consts.tile([P, MT2, D], BF16)
            nc.sync.dma_start(out=xi_sb[:], in_=xi_d.rearrange("(mt p) d -> p mt d", p=P))
            xit_sb = consts.tile([P, DT, ML], BF16)
            nc.sync.dma_start(out=xit_sb[:], in_=xit_d.rearrange("(dt p) m -> p dt m", p=P))
            sel_sb = consts.tile([P, NT, NL], BF16)
            nc.sync.dma_start(out=sel_sb[:], in_=sel_d.rearrange("(nt p) c -> p nt c", p=P))
            sel32 = consts.tile([P, NT, NL], F32)
            nc.vector.tensor_copy(out=sel32[:], in_=sel_sb[:])
            x_sb = consts.tile([P, NT, D], F32)
            nc.sync.dma_start(out=x_sb[:], in_=x_in.rearrange("(nt p) d -> p nt d", p=P))

            from concourse.masks import make_identity

            ident_f = consts.tile([P, P], F32)
            make_identity(nc, ident_f[:])
            ident_b = consts.tile([P, P], BF16)
            nc.vector.tensor_copy(out=ident_b[:], in_=ident_f[:])
            eps_t = consts.tile([P, 1], F32)
            nc.vector.memset(eps_t[:], EPS)
            zl_t = consts.tile([1, P], BF16)
            nc.vector.memset(zl_t[:], 0.0)
            zr_t = consts.tile([1, N], BF16)
            nc.vector.memset(zr_t[:], 0.0)

            for step in range(steps):
                # ======== LayerNorm forward (full tokens) ========
                xhat = work.tile([P, NT, D], BF16, tag="xhat")
                rr_t = stats.tile([P, NT], F32, tag="rr")
                for nt in range(NT):
                    xt = x_sb[:, nt, :]
                    st = stats.tile([P, 3, 6], F32, tag="bnst")
                    xg = xt.rearrange("p (g s) -> p g s", s=256)
                    for gs in range(3):
                        nc.vector.bn_stats(out=st[:, gs, :], in_=xg[:, gs, :])
                    mv = stats.tile([P, 2], F32, tag="mv")
                    nc.vector.bn_aggr(out=mv[:], in_=st[:])
                    rrx = rr_t[:, nt : nt + 1]
                    nc.scalar.activation(out=rrx, in_=mv[:, 1:2], func=AF.Sqrt, bias=eps_t[:], scale=1.0)
                    nc.vector.reciprocal(out=rrx, in_=rrx)
                    nmu = stats.tile([P, 1], F32, tag="nmu")
                    nc.vector.scalar_tensor_tensor(
                        out=nmu[:], in0=mv[:, 0:1], scalar=-1.0, in1=rrx, op0=OP.mult, op1=OP.mult,
                    )
                    nc.vector.tensor_scalar(
                        out=xhat[:, nt, :], in0=xt, scalar1=rrx, scalar2=nmu[:],
                        op0=OP.mult, op1=OP.add,
                    )

                # ======== gT = xhat^T (full, nt-major for LN overlap) ========
                gT = work.tile([P, DT, N], BF16, tag="gT")
                for nt in range(NT):
                    pg = ps2.tile([P, D], BF16, tag="ps2")
                    for dt in range(DT):
                        nc.tensor.transpose(pg[:, dt * P : (dt + 1) * P], xhat[:, nt, dt * P : (dt + 1) * P], ident_b[:])
                    nc.vector.tensor_copy(
                        out=gT[:, :, nt * P : (nt + 1) * P],
                        in_=pg[:].rearrange("p (dt n) -> p dt n", n=P),
                    )
                if debug_dump and step == 0:
                    dump("xhat", xhat[:])
                    dump("gT", gT[:])

                # ======== own-token selection (via sel matmuls) ========
                xhat_own = work.tile([P, OC, D], F32, tag="xh_own")
                xhat_own_b = work.tile([P, OC, D], BF16, tag="ocd_b")
                for oc in range(OC):
                    pa = pw.tile([P, 512], F32, tag="pw")
                    pb = ps2.tile([P, 512], F32, tag="ps2")
                    for nt in range(NT):
                        lh = sel_sb[:, nt, oc * P : (oc + 1) * P]
                        nc.tensor.matmul(pa[:], lh, xhat[:, nt, 0:512], start=(nt == 0), stop=(nt == NT - 1))
                        nc.tensor.matmul(pb[:, :256], lh, xhat[:, nt, 512:768], start=(nt == 0), stop=(nt == NT - 1))
                    nc.vector.tensor_copy(out=xhat_own[:, oc, 0:512], in_=pa[:])
                    nc.vector.tensor_copy(out=xhat_own[:, oc, 512:768], in_=pb[:, :256])
                    nc.vector.tensor_copy(out=xhat_own_b[:, oc, 0:512], in_=pa[:])
                    nc.vector.tensor_copy(out=xhat_own_b[:, oc, 512:768], in_=pb[:, :256])
                rstd_own = stats.tile([P, OC], F32, tag="rstd_own")
                for oc in range(OC):
                    pr = ps2.tile([P, 1], F32, tag="ps2")
                    for nt in range(NT):
                        nc.tensor.matmul(
                            pr[:], sel32[:, nt, oc * P : (oc + 1) * P], rr_t[:, nt : nt + 1],
                            start=(nt == 0), stop=(nt == NT - 1),
                        )
                    nc.vector.tensor_copy(out=rstd_own[:, oc : oc + 1], in_=pr[:])
                gT_own = work.tile([P, DT, NL], BF16, tag="gT_own")
                for dt in range(DT):
                    pg = ps2.tile([P, NL], BF16, tag="ps2")
                    for oc in range(OC):
                        nc.tensor.transpose(pg[:, oc * P : (oc + 1) * P], xhat_own_b[:, oc, dt * P : (dt + 1) * P], ident_b[:])
                    nc.vector.tensor_copy(out=gT_own[:, dt, :], in_=pg[:])
                if debug_dump and step == 0:
                    dump("xh_own", xhat_own[:])
                    nc.sync.dma_start(
                        out=dbg["rstd_own"].rearrange("(c p) o -> p c o", p=P),
                        in_=rstd_own[:].rearrange("p (c o) -> p c o", o=1),
                    )

                # ======== Hopfield (own memories, all tokens) ========
                RTs = rtp.tile([P, MT2, N], BF16, tag="RT")
                for mt in range(MT2):
                    hp = ps2.tile([P, N], F32, tag="ps2")
                    for dt in range(DT):
                        nc.tensor.matmul(
                            hp[:], xit_sb[:, dt, mt * P : (mt + 1) * P], gT[:, dt, :],
                            start=(dt == 0), stop=(dt == DT - 1),
                        )
                    nc.vector.tensor_scalar(
                        out=RTs[:, mt, :], in0=hp[:], scalar1=0.0, scalar2=None, op0=OP.max,
                    )

                # ======== projections ========
                q_own = work.tile([P, OC, EW], BF16, tag="q_own")
                for oc in range(OC):
                    pa = pw.tile([P, 512], F32, tag="pw")
                    pb = ps2.tile([P, 512], F32, tag="ps2")
                    for dt in range(DT):
                        lh = gT_own[:, dt, oc * P : (oc + 1) * P]
                        nc.tensor.matmul(pa[:, :384], lh, wq_sb[:, dt, 0:384], start=(dt == 0), stop=(dt == DT - 1))
                        nc.tensor.matmul(pb[:, :384], lh, wq_sb[:, dt, 384:768], start=(dt == 0), stop=(dt == DT - 1))
                    nc.vector.tensor_copy(out=q_own[:, oc, 0:384], in_=pa[:, :384])
                    nc.vector.tensor_copy(out=q_own[:, oc, 384:768], in_=pb[:, :384])
                qT = work.tile([P, ET, NL], BF16, tag="qT")
                for et in range(ET):
                    pg = ps2.tile([P, NL], BF16, tag="ps2")
                    for oc in range(OC):
                        nc.tensor.transpose(pg[:, oc * P : (oc + 1) * P], q_own[:, oc, et * P : (et + 1) * P], ident_b[:])
                    nc.vector.tensor_copy(out=qT[:, et, :], in_=pg[:])
                kT = work.tile([P, ET, N], BF16, tag="kT")
                for et in range(ET):
                    pa = pw.tile([P, 512], F32, tag="pw")
                    for dt in range(DT):
                        nc.tensor.matmul(
                            pa[:], wk_sb[:, dt, et * P : (et + 1) * P], gT[:, dt, :],
                            start=(dt == 0), stop=(dt == DT - 1),
                        )
                    nc.vector.tensor_copy(out=kT[:, et, :], in_=pa[:])
                k_sb = work.tile([P, NT, EW], BF16, tag="k")
                for nt in range(NT):
                    pg = ps2.tile([P, EW], BF16, tag="ps2")
                    for et in range(ET):
                        nc.tensor.transpose(pg[:, et * P : (et + 1) * P], kT[:, et, nt * P : (nt + 1) * P], ident_b[:])
                    nc.vector.tensor_copy(
                        out=k_sb[:, nt, :], in_=pg[:],
                    )
                if debug_dump and step == 0:
                    dump("q_own", q_own[:])
                    dump("kT", kT[:])

                # ======== attention heads ========
                dqTst = work.tile([P, ET, NL], BF16, tag="dqTst")
                dkTst = work.tile([P, ET, N], BF16, tag="dkTst")
                pq = pk = None
                for h in range(H):
                    et, eo = h // 2, (h % 2) * HD
                    Un = upool.tile([P, OC, N], BF16, tag="Un")
                    sm = stats.tile([P, OC], F32, tag="sm")
                    for oc in range(OC):
                        sc = ps2.tile([P, 512], F32, tag="ps2")
                        nc.tensor.matmul(
                            sc[:], qT[eo : eo + HD, et, oc * P : (oc + 1) * P],
                            kT[eo : eo + HD, et, :], start=True, stop=True,
                        )
                        nc.scalar.activation(
                            out=Un[:, oc, :], in_=sc[:], func=AF.Exp, bias=0.0, scale=1.0,
                            accum_out=sm[:, oc : oc + 1],
                        )
                    nc.vector.reciprocal(out=sm[:], in_=sm[:])
                    dg_m = stats.tile([P, OC, P], BF16, tag="diag")
                    for oc in range(OC):
                        nc.vector.tensor_scalar_mul(out=dg_m[:, oc, :], in0=ident_b[:], scalar1=sm[:, oc : oc + 1])
                        nc.vector.tensor_scalar_mul(
                            out=q_own[:, oc, h * HD : (h + 1) * HD],
                            in0=q_own[:, oc, h * HD : (h + 1) * HD], scalar1=sm[:, oc : oc + 1],
                        )
                    if debug_dump and step == 0 and h == 0:
                        dump("U0", Un[:])
                    PTn = ptool.tile([P, NT, NL], BF16, tag="PTn")
                    for mt in range(NT):
                        pp = ps2.tile([P, NL], F32, tag="ps2")
                        for oc in range(OC):
                            nc.tensor.matmul(
                                pp[:, oc * P : (oc + 1) * P], Un[:, oc, mt * P : (mt + 1) * P],
                                dg_m[:, oc, :], start=True, stop=True,
                            )
                        nc.vector.tensor_copy(out=PTn[:, mt, :], in_=pp[:])
                    if debug_dump and step == 0 and h == 0:
                        dump("PT0", PTn[:])
                    if h % 2 == 0:
                        pq = pw.tile([P, NL], F32, tag="pw")
                        pk = pw.tile([P, N], F32, tag="pw")
                    for mt in range(NT):
                        nc.tensor.matmul(
                            pq[eo : eo + HD, :], k_sb[:, mt, h * HD : (h + 1) * HD], PTn[:, mt, :],
                            start=(mt == 0), stop=(mt == NT - 1),
                        )
                    for oc in range(OC):
                        nc.tensor.matmul(
                            pk[eo : eo + HD, :], q_own[:, oc, h * HD : (h + 1) * HD], Un[:, oc, :],
                            start=(oc == 0), stop=(oc == OC - 1),
                        )
                    if h % 2 == 1:
                        nc.scalar.activation(out=dqTst[:, et, :], in_=pq[:], func=AF.Copy)
                        nc.scalar.activation(out=dkTst[:, et, :], in_=pk[:], func=AF.Copy)

                if debug_dump and step == 0:
                    dump("dqTst", dqTst[:])
                    dump("dkTst", dkTst[:])

                # ======== pair ReduceScatter of dk^T ========
                dkT_own = work.tile([P, ET, NL], BF16, tag="dkT_own")
                if with_cc:
                    rs_in = drp.tile([2, EW, NL], BF16, tag="rs_in")
                    rs_out = drp.tile([EW, NL], BF16, tag="rs_out")
                    for r in range(2):
                        nc.sync.dma_start(
                            out=rs_in[r].rearrange("(et p) n -> p et n", p=P),
                            in_=dkTst[:, :, r * NL : (r + 1) * NL],
                        )
                    nc.gpsimd.collective_compute(
                        "ReduceScatter", OP.add, replica_groups=REPLICA_GROUPS,
                        ins=[rs_in.opt()], outs=[rs_out.opt()],
                    )
                    nc.sync.dma_start(out=dkT_own[:], in_=rs_out.rearrange("(et p) n -> p et n", p=P))
                else:
                    nc.vector.tensor_copy(out=dkT_own[:], in_=dkTst[:, :, 0:NL])
                if debug_dump and step == 0:
                    dump("dkT_own", dkT_own[:])

                # ======== Hopfield bwd (covers RS1), then RS2 ========
                dgh_s = work.tile([P, DT, N], BF16, tag="dgh_s")
                hop_dr = drp.tile([2, D, NL], BF16, tag="rs2_in")
                for dh in range(2):
                    hb = [pacc.tile([P, N], F32, tag=f"c{b}", name=f"hop{dh}{b}") for b in range(3)]
                    for mt in range(MT2):
                        for b in range(3):
                            dt = dh * 3 + b
                            nc.tensor.matmul(
                                hb[b][:], xi_sb[:, mt, dt * P : (dt + 1) * P], RTs[:, mt, :],
                                start=(mt == 0), stop=(mt == MT2 - 1),
                            )
                    for b in range(3):
                        nc.vector.tensor_copy(out=dgh_s[:, dh * 3 + b, :], in_=hb[b][:])
                for r in range(2):
                    nc.sync.dma_start(
                        out=hop_dr[r].rearrange("(dt p) n -> p dt n", p=P),
                        in_=dgh_s[:, :, r * NL : (r + 1) * NL],
                    )
                hopT_own = work.tile([P, DT, NL], BF16, tag="hopT_own")
                if with_cc:
                    rs2_out = drp.tile([D, NL], BF16, tag="rs2_out")
                    nc.gpsimd.collective_compute(
                        "ReduceScatter", OP.add, replica_groups=REPLICA_GROUPS,
                        ins=[hop_dr.opt()], outs=[rs2_out.opt()],
                    )
                    nc.sync.dma_start(out=hopT_own[:], in_=rs2_out.rearrange("(dt p) n -> p dt n", p=P))
                else:
                    nc.sync.dma_start(out=hopT_own[:], in_=hop_dr[0].rearrange("(dt p) n -> p dt n", p=P))
                if debug_dump and step == 0:
                    dump("hopT_own", hopT_own[:])

                # ======== attention dgT accumulation (3 pacc banks) ========
                dgTb = [pacc.tile([P, N], F32, tag=f"c{b}", name=f"dga{b}") for b in range(3)]
                for b in range(3):
                    nc.tensor.matmul(dgTb[b][:], zl_t[:], zr_t[:], start=True, stop=False)
                for dt in range(DT):
                    b, half = dt // 2, dt % 2
                    for et in range(ET):
                        nc.tensor.matmul(
                            dgTb[b][:, half * NL : (half + 1) * NL],
                            wqt_sb[:, et, dt * P : (dt + 1) * P], dqTst[:, et, :],
                            start=False, stop=False,
                        )
                for dt in range(DT):
                    b, half = dt // 2, dt % 2
                    for et in range(ET):
                        nc.tensor.matmul(
                            dgTb[b][:, half * NL : (half + 1) * NL],
                            wkt_sb[:, et, dt * P : (dt + 1) * P], dkT_own[:, et, :],
                            start=False, stop=(et == ET - 1 and half == 1),
                        )

                # ======== combine + transpose + LayerNorm backward ========
                dgTs = work.tile([P, DT, NL], BF16, tag="qT")
                for b in range(3):
                    nc.vector.tensor_tensor(
                        out=dgTs[:, 2 * b : 2 * b + 2, :].rearrange("p t n -> p (t n)"),
                        in0=dgTb[b][:],
                        in1=hopT_own[:, 2 * b : 2 * b + 2, :].rearrange("p t n -> p (t n)"),
                        op=OP.add,
                    )
                if debug_dump and step == 0:
                    dump("dgTs", dgTs[:])
                dg_own = work.tile([P, OC, D], F32, tag="dg_own")
                dxb = work.tile([P, OC, D], BF16, tag="ocd_b")
                m1s = stats.tile([P, OC], F32, tag="m1s")
                u2s = stats.tile([P, OC], F32, tag="u2s")
                for oc in range(OC):
                    pg = ps2.tile([P, D], BF16, tag="ps2")
                    for dt in range(DT):
                        nc.tensor.transpose(pg[:, dt * P : (dt + 1) * P], dgTs[:, dt, oc * P : (oc + 1) * P], ident_b[:])
                    nc.vector.scalar_tensor_tensor(
                        out=dg_own[:, oc, :], in0=pg[:], scalar=rstd_own[:, oc : oc + 1],
                        in1=xhat_own[:, oc, :], op0=OP.mult, op1=OP.bypass,
                        accum_out=m1s[:, oc : oc + 1],
                    )
                    prod = work.tile([P, D], F32, tag="prod")
                    nc.vector.scalar_tensor_tensor(
                        out=prod[:], in0=dg_own[:, oc, :], scalar=1.0, in1=xhat_own[:, oc, :],
                        op0=OP.mult, op1=OP.mult, accum_out=u2s[:, oc : oc + 1],
                    )
                s01 = stats.tile([P, OC, 2], F32, tag="s01")
                nc.vector.tensor_scalar(
                    out=s01[:, :, 0], in0=u2s[:], scalar1=1.0 / D, scalar2=None, op0=OP.mult,
                )
                nc.vector.tensor_scalar(
                    out=s01[:, :, 1], in0=m1s[:], scalar1=1.0 / D, scalar2=None, op0=OP.mult,
                )
                for oc in range(OC):
                    nc.vector.ln_bwd_dx(
                        out=dxb[:, oc, :], dy=dg_own[:, oc, :], x_hat=xhat_own[:, oc, :],
                        mean_dyx=s01[:, oc, 0:1], mean_dy=s01[:, oc, 1:2], scale=ALPHA,
                    )
                if debug_dump and step == 0:
                    dump("dg_own", dg_own[:])
                    dump("s01", s01[:])
                    dump("dxb", dxb[:])

                # ======== pair AllGather of dx (chunked); update x ========
                dxg = work.tile([P, NT, D], BF16, tag="k")
                if with_cc:
                    for oc in range(OC):
                        ag_in = drp.tile([P, D], BF16, tag=f"ag_in{oc}")
                        ag_out = drp.tile([2 * P, D], BF16, tag=f"ag_out{oc}")
                        nc.sync.dma_start(out=ag_in[:], in_=dxb[:, oc, :])
                        nc.gpsimd.collective_compute(
                            "AllGather", OP.bypass, replica_groups=REPLICA_GROUPS,
                            ins=[ag_in.opt()], outs=[ag_out.opt()],
                        )
                        # AG(oc) delivers true chunks {oc, 2+oc}
                        for r in range(2):
                            nt = 2 * r + oc
                            nc.sync.dma_start(out=dxg[:, nt, :], in_=ag_out[r * P : (r + 1) * P, :])
                            nc.vector.scalar_tensor_tensor(
                                out=x_sb[:, nt, :], in0=dxg[:, nt, :], scalar=1.0, in1=x_sb[:, nt, :],
                                op0=OP.mult, op1=OP.add,
                            )
                else:
                    nc.vector.memset(dxg[:], 0.0)
                    nc.vector.tensor_copy(
                        out=dxg[:, 0:OC, :].rearrange("p t d -> p (t d)"),
                        in_=dxb[:].rearrange("p t d -> p (t d)"),
                    )
                    for nt in range(NT):
                        nc.vector.scalar_tensor_tensor(
                            out=x_sb[:, nt, :], in0=dxg[:, nt, :], scalar=1.0, in1=x_sb[:, nt, :],
                            op0=OP.mult, op1=OP.add,
                        )

            for nt in range(NT):
                nc.sync.dma_start(out=x_out[nt * P : (nt + 1) * P, :], in_=x_sb[:, nt, :])

    nc.compile()
    return nc


def _prep_inputs(x, gamma, delta, Wq, Wk, xi):
    """Build the 8 per-core input dicts (host-side sharding + weight folding)."""
    assert np.allclose(delta, 0.0), "kernel requires delta == 0"
    import ml_dtypes

    bf = ml_dtypes.bfloat16
    beta_sqrt = np.float32(1.0 / np.sqrt(np.sqrt(np.float32(HD))))
    g = gamma.astype(np.float32)
    wq = ((Wq * g[None, :, None]).transpose(1, 0, 2).reshape(D, EW) * beta_sqrt).astype(bf)
    wk = ((Wk * g[None, :, None]).transpose(1, 0, 2).reshape(D, EW) * beta_sqrt).astype(bf)
    wqt = ((Wq * g[None, :, None]).transpose(0, 2, 1).reshape(EW, D) / beta_sqrt).astype(bf)
    wkt = ((Wk * g[None, :, None]).transpose(0, 2, 1).reshape(EW, D) / beta_sqrt).astype(bf)
    xi_f = (xi * g[None, :]).astype(np.float32)
    sels, xis, xits = [], [], []
    for j in range(2):
        s = np.zeros((N, NL), dtype=bf)
        s[np.arange(j * NL, (j + 1) * NL), np.arange(NL)] = 1
        sels.append(s)
        sh = xi_f[j * ML : (j + 1) * ML]
        xis.append(np.ascontiguousarray(sh).astype(bf))
        xits.append(np.ascontiguousarray(sh.T).astype(bf))
    in_maps = []
    for c in range(8):
        b, j = c // 2, c % 2
        in_maps.append(
            {
                "x": np.ascontiguousarray(x[b]),
                "sel": sels[j],
                "wq": wq, "wk": wk, "wqt": wqt, "wkt": wkt,
                "xi": xis[j], "xit": xits[j],
            }
        )
    return in_maps


_NC_CACHE = {}


def _get_nc(steps=STEPS, with_cc=True):
    key = (steps, with_cc)
    if key not in _NC_CACHE:
        _NC_CACHE[key] = build_kernel(steps, with_cc)
    return _NC_CACHE[key]


def kernel(x, gamma, delta, Wq, Wk, xi):
    from concourse.bass_utils import run_bass_kernel_spmd

    x = np.asarray(x, dtype=np.float32)
    in_maps = _prep_inputs(
        x,
        np.asarray(gamma, np.float32),
        np.asarray(delta, np.float32),
        np.asarray(Wq, np.float32),
        np.asarray(Wk, np.float32),
        np.asarray(xi, np.float32),
    )
    nc = _get_nc()
    res = run_bass_kernel_spmd(nc, in_maps, list(range(8)))
    out = np.stack([res.results[2 * b]["x_out"] for b in range(B)], axis=0)
    return out.astype(np.float32)


# revision 21
# speedup vs baseline: 1.1833x; 1.0192x over previous
"""Energy Transformer descent kernel for 8 Trainium2 NeuronCores.

Problem: 12 steps of gradient descent on
  E(x) = -(1/beta) sum logsumexp(beta q k^T) - 0.5 sum relu(g xi^T)^2,
  g = LayerNorm(x; gamma, delta), q = g Wq_h, k = g Wk_h.

Sharding: data-parallel over batch B=4 -> core pairs (2b, 2b+1); within a
pair core j owns TOKENS j*256..(j+1)*256 for the attention-query /
LayerNorm-backward work and MEMORIES j*1536..(j+1)*1536 for the Hopfield
term (full-width 512-token matmuls there).  Three pair-collectives per
step, two of them overlapped with compute:
  RS2: ReduceScatter(dg_hop [D, N] f32) -> own-token columns; issued right
       after the Hopfield phase, lands during projections+attention.
  RS1: ReduceScatter(dk^T [EW, N] bf16) -> own-token columns; issued after
       the head loop.
  AG:  AllGather(dx own-half) -> full dx for the x update (exposed).

The SPMD program is identical on both cores: token/memory ownership enters
only through per-core inputs (a one-hot selection matrix `sel` and the
xi/xit shards) and the rank-major layouts of the collective buffers.

Host-side folding (delta must be zero, which the problem guarantees):
  wq = sqrt(beta) diag(gamma) Wq   (likewise wk)
  wqt = (1/sqrt(beta)) (diag(gamma) Wq)^T   (likewise wkt)
  xi' = xi diag(gamma)
All matmuls run in bf16 (fp32 PSUM accumulation); fp8 was measured to
break the 2e-2 gate.  Softmax normalisation is folded into the P-transpose
by multiplying with diag(1/rowsum) instead of the identity, and into the
dk^T matmul by pre-scaling q rows.  PSUM plan (8 banks): pw 2 + ps2 3 +
pacc 3, where pacc holds the Hopfield-bwd chains (two passes over d-chunk
halves) and is then reused for the attention dgT chains (zero-init via a
dummy matmul; interleaved half-chains must never each use start=True).
"""

import numpy as np

import concourse.bass as bass
import concourse.tile as tile
from concourse import bacc, mybir

STEPS = 12
ALPHA = 0.125
EPS = 1e-5
B, N, D, H, HD, M = 4, 512, 768, 12, 64, 3072
P = 128
NT = N // P   # 4 full-token chunks
OC = 2        # own-token chunks
NL = OC * P   # 256 own tokens
DT = D // P   # 6 embed chunks
EW = H * HD   # 768 head width (all heads)
ET = EW // P  # 6 head-dim chunks
ML = M // 2   # 1536 own memories
MT2 = ML // P  # 12 own memory chunks
F32 = mybir.dt.float32
BF16 = mybir.dt.bfloat16
AF = mybir.ActivationFunctionType
OP = mybir.AluOpType

REPLICA_GROUPS = [[0, 1], [2, 3], [4, 5], [6, 7]]


def build_kernel(steps=STEPS, with_cc=True, debug_dump=False):
    nc = bacc.Bacc("TRN2", target_bir_lowering=False, debug=False, num_devices=8)

    x_in = nc.declare_dram_parameter("x", [N, D], F32, isOutput=False)
    sel_d = nc.declare_dram_parameter("sel", [N, NL], BF16, isOutput=False)
    wq_d = nc.declare_dram_parameter("wq", [D, EW], BF16, isOutput=False)
    wk_d = nc.declare_dram_parameter("wk", [D, EW], BF16, isOutput=False)
    wqt_d = nc.declare_dram_parameter("wqt", [EW, D], BF16, isOutput=False)
    wkt_d = nc.declare_dram_parameter("wkt", [EW, D], BF16, isOutput=False)
    xi_d = nc.declare_dram_parameter("xi", [ML, D], BF16, isOutput=False)
    xit_d = nc.declare_dram_parameter("xit", [D, ML], BF16, isOutput=False)
    x_out = nc.declare_dram_parameter("x_out", [N, D], F32, isOutput=True)
    dbg = {}
    if debug_dump:
        for nm, shp, dt_ in (("xhat", [N, D], BF16), ("xh_own", [NL, D], F32),
                             ("gT", [D, N], BF16), ("hopT_own", [D, NL], BF16),
                             ("q_own", [NL, EW], BF16), ("kT", [EW, N], BF16),
                             ("U0", [NL, N], BF16), ("PT0", [N, NL], BF16),
                             ("dqTst", [EW, NL], BF16), ("dkTst", [EW, N], BF16),
                             ("dkT_own", [EW, NL], BF16), ("dgTs", [D, NL], BF16),
                             ("dg_own", [NL, D], F32), ("rstd_own", [NL, 1], F32),
                             ("s01", [NL, 2], F32), ("dxb", [NL, D], BF16)):
            dbg[nm] = nc.declare_dram_parameter("o_" + nm, shp, dt_, isOutput=True)

    def dump(nm, ap, pdim=P):
        if debug_dump:
            nc.sync.dma_start(out=dbg[nm].rearrange("(a p) b -> p a b", p=pdim), in_=ap)

    with tile.TileContext(nc) as tc:
        import contextlib

        with contextlib.ExitStack() as ctx:
            consts = ctx.enter_context(tc.tile_pool(name="consts", bufs=1))
            work = ctx.enter_context(tc.tile_pool(name="work", bufs=1))
            upool = ctx.enter_context(tc.tile_pool(name="upool", bufs=2))
            ptool = ctx.enter_context(tc.tile_pool(name="ptool", bufs=2))
            rtp = ctx.enter_context(tc.tile_pool(name="rtp", bufs=1))
            stats = ctx.enter_context(tc.tile_pool(name="stats", bufs=4))
            # PSUM: pw 2 + ps2 3 + pacc 3 = 8 banks
            pw = ctx.enter_context(tc.tile_pool(name="pw", bufs=2, space="PSUM"))
            ps2 = ctx.enter_context(tc.tile_pool(name="ps2", bufs=3, space="PSUM"))
            pacc = ctx.enter_context(tc.tile_pool(name="pacc", bufs=1, space="PSUM"))
            drp = ctx.enter_context(tc.tile_pool(name="drp", bufs=2, space="DRAM"))

            # ---- resident tensors ----
            wq_sb = consts.tile([P, DT, EW], BF16)
            nc.sync.dma_start(out=wq_sb[:], in_=wq_d.rearrange("(dt p) e -> p dt e", p=P))
            wk_sb = consts.tile([P, DT, EW], BF16)
            nc.sync.dma_start(out=wk_sb[:], in_=wk_d.rearrange("(dt p) e -> p dt e", p=P))
            wqt_sb = consts.tile([P, ET, D], BF16)
            nc.sync.dma_start(out=wqt_sb[:], in_=wqt_d.rearrange("(et p) d -> p et d", p=P))
            wkt_sb = consts.tile([P, ET, D], BF16)
            nc.sync.dma_start(out=wkt_sb[:], in_=wkt_d.rearrange("(et p) d -> p et d", p=P))
            xi_sb = consts.tile([P, MT2, D], BF16)
            nc.sync.dma_start(out=xi_sb[:], in_=xi_d.rearrange("(mt p) d -> p mt d", p=P))
            xit_sb = consts.tile([P, DT, ML], BF16)
            nc.sync.dma_start(out=xit_sb[:], in_=xit_d.rearrange("(dt p) m -> p dt m", p=P))
            sel_sb = consts.tile([P, NT, NL], BF16)
            nc.sync.dma_start(out=sel_sb[:], in_=sel_d.rearrange("(nt p) c -> p nt c", p=P))
            sel32 = consts.tile([P, NT, NL], F32)
            nc.vector.tensor_copy(out=sel32[:], in_=sel_sb[:])
            x_sb = consts.tile([P, NT, D], F32)
            nc.sync.dma_start(out=x_sb[:], in_=x_in.rearrange("(nt p) d -> p nt d", p=P))

            from concourse.masks import make_identity

            ident_f = consts.tile([P, P], F32)
            make_identity(nc, ident_f[:])
            ident_b = consts.tile([P, P], BF16)
            nc.vector.tensor_copy(out=ident_b[:], in_=ident_f[:])
            eps_t = consts.tile([P, 1], F32)
            nc.vector.memset(eps_t[:], EPS)
            zl_t = consts.tile([1, P], BF16)
            nc.vector.memset(zl_t[:], 0.0)
            zr_t = consts.tile([1, N], BF16)
            nc.vector.memset(zr_t[:], 0.0)

            for step in range(steps):
                # ======== LayerNorm forward (full tokens) ========
                xhat = work.tile([P, NT, D], BF16, tag="xhat")
                rr_t = stats.tile([P, NT], F32, tag="rr")
                for nt in range(NT):
                    xt = x_sb[:, nt, :]
                    st = stats.tile([P, 3, 6], F32, tag="bnst")
                    xg = xt.rearrange("p (g s) -> p g s", s=256)
                    for gs in range(3):
                        nc.vector.bn_stats(out=st[:, gs, :], in_=xg[:, gs, :])
                    mv = stats.tile([P, 2], F32, tag="mv")
                    nc.vector.bn_aggr(out=mv[:], in_=st[:])
                    rrx = rr_t[:, nt : nt + 1]
                    nc.scalar.activation(out=rrx, in_=mv[:, 1:2], func=AF.Sqrt, bias=eps_t[:], scale=1.0)
                    nc.vector.reciprocal(out=rrx, in_=rrx)
                    nmu = stats.tile([P, 1], F32, tag="nmu")
                    nc.vector.scalar_tensor_tensor(
                        out=nmu[:], in0=mv[:, 0:1], scalar=-1.0, in1=rrx, op0=OP.mult, op1=OP.mult,
                    )
                    nc.vector.tensor_scalar(
                        out=xhat[:, nt, :], in0=xt, scalar1=rrx, scalar2=nmu[:],
                        op0=OP.mult, op1=OP.add,
                    )

                # ======== gT = xhat^T (full, nt-major for LN overlap) ========
                gT = work.tile([P, DT, N], BF16, tag="gT")
                for nt in range(NT):
                    pg = ps2.tile([P, D], BF16, tag="ps2")
                    for dt in range(DT):
                        nc.tensor.transpose(pg[:, dt * P : (dt + 1) * P], xhat[:, nt, dt * P : (dt + 1) * P], ident_b[:])
                    nc.vector.tensor_copy(
                        out=gT[:, :, nt * P : (nt + 1) * P],
                        in_=pg[:].rearrange("p (dt n) -> p dt n", n=P),
                    )
                if debug_dump and step == 0:
                    dump("xhat", xhat[:])
                    dump("gT", gT[:])

                # ======== own-token selection (via sel matmuls) ========
                xhat_own = work.tile([P, OC, D], F32, tag="xh_own")
                xhat_own_b = work.tile([P, OC, D], BF16, tag="ocd_b")
                for oc in range(OC):
                    pa = pw.tile([P, 512], F32, tag="pw")
                    pb = ps2.tile([P, 512], F32, tag="ps2")
                    for nt in range(NT):
                        lh = sel_sb[:, nt, oc * P : (oc + 1) * P]
                        nc.tensor.matmul(pa[:], lh, xhat[:, nt, 0:512], start=(nt == 0), stop=(nt == NT - 1))
                        nc.tensor.matmul(pb[:, :256], lh, xhat[:, nt, 512:768], start=(nt == 0), stop=(nt == NT - 1))
                    nc.vector.tensor_copy(out=xhat_own[:, oc, 0:512], in_=pa[:])
                    nc.vector.tensor_copy(out=xhat_own[:, oc, 512:768], in_=pb[:, :256])
                    nc.vector.tensor_copy(out=xhat_own_b[:, oc, 0:512], in_=pa[:])
                    nc.vector.tensor_copy(out=xhat_own_b[:, oc, 512:768], in_=pb[:, :256])
                rstd_own = stats.tile([P, OC], F32, tag="rstd_own")
                for oc in range(OC):
                    pr = ps2.tile([P, 1], F32, tag="ps2")
                    for nt in range(NT):
                        nc.tensor.matmul(
                            pr[:], sel32[:, nt, oc * P : (oc + 1) * P], rr_t[:, nt : nt + 1],
                            start=(nt == 0), stop=(nt == NT - 1),
                        )
                    nc.vector.tensor_copy(out=rstd_own[:, oc : oc + 1], in_=pr[:])
                gT_own = work.tile([P, DT, NL], BF16, tag="gT_own")
                for dt in range(DT):
                    pg = ps2.tile([P, NL], BF16, tag="ps2")
                    for oc in range(OC):
                        nc.tensor.transpose(pg[:, oc * P : (oc + 1) * P], xhat_own_b[:, oc, dt * P : (dt + 1) * P], ident_b[:])
                    nc.vector.tensor_copy(out=gT_own[:, dt, :], in_=pg[:])
                if debug_dump and step == 0:
                    dump("xh_own", xhat_own[:])
                    nc.sync.dma_start(
                        out=dbg["rstd_own"].rearrange("(c p) o -> p c o", p=P),
                        in_=rstd_own[:].rearrange("p (c o) -> p c o", o=1),
                    )

                # ======== Hopfield (own memories, all tokens) ========
                RTs = rtp.tile([P, MT2, N], BF16, tag="RT")
                for mt in range(MT2):
                    hp = ps2.tile([P, N], F32, tag="ps2")
                    for dt in range(DT):
                        nc.tensor.matmul(
                            hp[:], xit_sb[:, dt, mt * P : (mt + 1) * P], gT[:, dt, :],
                            start=(dt == 0), stop=(dt == DT - 1),
                        )
                    nc.vector.tensor_scalar(
                        out=RTs[:, mt, :], in0=hp[:], scalar1=0.0, scalar2=None, op0=OP.max,
                    )

                # ======== projections ========
                q_own = work.tile([P, OC, EW], BF16, tag="q_own")
                for oc in range(OC):
                    pa = pw.tile([P, 512], F32, tag="pw")
                    pb = ps2.tile([P, 512], F32, tag="ps2")
                    for dt in range(DT):
                        lh = gT_own[:, dt, oc * P : (oc + 1) * P]
                        nc.tensor.matmul(pa[:, :384], lh, wq_sb[:, dt, 0:384], start=(dt == 0), stop=(dt == DT - 1))
                        nc.tensor.matmul(pb[:, :384], lh, wq_sb[:, dt, 384:768], start=(dt == 0), stop=(dt == DT - 1))
                    nc.vector.tensor_copy(out=q_own[:, oc, 0:384], in_=pa[:, :384])
                    nc.vector.tensor_copy(out=q_own[:, oc, 384:768], in_=pb[:, :384])
                qT = work.tile([P, ET, NL], BF16, tag="qT")
                for et in range(ET):
                    pg = ps2.tile([P, NL], BF16, tag="ps2")
                    for oc in range(OC):
                        nc.tensor.transpose(pg[:, oc * P : (oc + 1) * P], q_own[:, oc, et * P : (et + 1) * P], ident_b[:])
                    nc.vector.tensor_copy(out=qT[:, et, :], in_=pg[:])
                kT = work.tile([P, ET, N], BF16, tag="kT")
                for et in range(ET):
                    pa = pw.tile([P, 512], F32, tag="pw")
                    for dt in range(DT):
                        nc.tensor.matmul(
                            pa[:], wk_sb[:, dt, et * P : (et + 1) * P], gT[:, dt, :],
                            start=(dt == 0), stop=(dt == DT - 1),
                        )
                    nc.vector.tensor_copy(out=kT[:, et, :], in_=pa[:])
                k_sb = work.tile([P, NT, EW], BF16, tag="k")
                for nt in range(NT):
                    pg = ps2.tile([P, EW], BF16, tag="ps2")
                    for et in range(ET):
                        nc.tensor.transpose(pg[:, et * P : (et + 1) * P], kT[:, et, nt * P : (nt + 1) * P], ident_b[:])
                    nc.vector.tensor_copy(
                        out=k_sb[:, nt, :], in_=pg[:],
                    )
                if debug_dump and step == 0:
                    dump("q_own", q_own[:])
                    dump("kT", kT[:])

                # ======== attention heads ========
                dqTst = work.tile([P, ET, NL], BF16, tag="dqTst")
                dkTst = work.tile([P, ET, N], BF16, tag="dkTst")
                pq = pk = None
                for h in range(H):
                    et, eo = h // 2, (h % 2) * HD
                    Un = upool.tile([P, OC, N], BF16, tag="Un")
                    sm = stats.tile([P, OC], F32, tag="sm")
                    for oc in range(OC):
                        sc = ps2.tile([P, 512], F32, tag="ps2")
                        nc.tensor.matmul(
                            sc[:], qT[eo : eo + HD, et, oc * P : (oc + 1) * P],
                            kT[eo : eo + HD, et, :], start=True, stop=True,
                        )
                        nc.scalar.activation(
                            out=Un[:, oc, :], in_=sc[:], func=AF.Exp, bias=0.0, scale=1.0,
                            accum_out=sm[:, oc : oc + 1],
                        )
                    nc.vector.reciprocal(out=sm[:], in_=sm[:])
                    dg_m = stats.tile([P, OC, P], BF16, tag="diag")
                    for oc in range(OC):
                        nc.vector.tensor_scalar_mul(out=dg_m[:, oc, :], in0=ident_b[:], scalar1=sm[:, oc : oc + 1])
                        nc.vector.tensor_scalar_mul(
                            out=q_own[:, oc, h * HD : (h + 1) * HD],
                            in0=q_own[:, oc, h * HD : (h + 1) * HD], scalar1=sm[:, oc : oc + 1],
                        )
                    if debug_dump and step == 0 and h == 0:
                        dump("U0", Un[:])
                    PTn = ptool.tile([P, NT, NL], BF16, tag="PTn")
                    for mt in range(NT):
                        pp = ps2.tile([P, NL], F32, tag="ps2")
                        for oc in range(OC):
                            nc.tensor.matmul(
                                pp[:, oc * P : (oc + 1) * P], Un[:, oc, mt * P : (mt + 1) * P],
                                dg_m[:, oc, :], start=True, stop=True,
                            )
                        nc.vector.tensor_copy(out=PTn[:, mt, :], in_=pp[:])
                    if debug_dump and step == 0 and h == 0:
                        dump("PT0", PTn[:])
                    if h % 2 == 0:
                        pq = pw.tile([P, NL], F32, tag="pw")
                        pk = pw.tile([P, N], F32, tag="pw")
                    for mt in range(NT):
                        nc.tensor.matmul(
                            pq[eo : eo + HD, :], k_sb[:, mt, h * HD : (h + 1) * HD], PTn[:, mt, :],
                            start=(mt == 0), stop=(mt == NT - 1),
                        )
                    for oc in range(OC):
                        nc.tensor.matmul(
                            pk[eo : eo + HD, :], q_own[:, oc, h * HD : (h + 1) * HD], Un[:, oc, :],
                            start=(oc == 0), stop=(oc == OC - 1),
                        )
                    if h % 2 == 1:
                        nc.scalar.activation(out=dqTst[:, et, :], in_=pq[:], func=AF.Copy)
                        nc.scalar.activation(out=dkTst[:, et, :], in_=pk[:], func=AF.Copy)
                    if h == 5 and with_cc:
                        # RS1a: first head-half's dk^T, overlaps heads 6-11
                        rs1a_in = drp.tile([2, EW // 2, NL], BF16, tag="rs1a_in")
                        rs1a_out = drp.tile([EW // 2, NL], BF16, tag="rs1a_out")
                        for r in range(2):
                            nc.sync.dma_start(
                                out=rs1a_in[r].rearrange("(et p) n -> p et n", p=P),
                                in_=dkTst[:, 0:3, r * NL : (r + 1) * NL],
                            )
                        nc.gpsimd.collective_compute(
                            "ReduceScatter", OP.add, replica_groups=REPLICA_GROUPS,
                            ins=[rs1a_in.opt()], outs=[rs1a_out.opt()],
                        )

                dkT_own = work.tile([P, ET, NL], BF16, tag="dkT_own")
                if with_cc:
                    nc.sync.dma_start(
                        out=dkT_own[:, 0:3, :], in_=rs1a_out.rearrange("(et p) n -> p et n", p=P),
                    )
                else:
                    nc.vector.tensor_copy(out=dkT_own[:], in_=dkTst[:, :, 0:NL])
                if debug_dump and step == 0:
                    dump("dqTst", dqTst[:])
                    dump("dkTst", dkTst[:])

                # ======== Hopfield bwd (covers RS1), then RS2 ========
                dgh_s = work.tile([P, DT, N], BF16, tag="dgh_s")
                for dh in range(2):
                    hb = [pacc.tile([P, N], F32, tag=f"c{b}", name=f"hop{dh}{b}") for b in range(3)]
                    for mt in range(MT2):
                        for b in range(3):
                            dt = dh * 3 + b
                            nc.tensor.matmul(
                                hb[b][:], xi_sb[:, mt, dt * P : (dt + 1) * P], RTs[:, mt, :],
                                start=(mt == 0), stop=(mt == MT2 - 1),
                            )
                    for b in range(3):
                        nc.vector.tensor_copy(out=dgh_s[:, dh * 3 + b, :], in_=hb[b][:])
                hopT_own = work.tile([P, DT, NL], BF16, tag="hopT_own")
                if with_cc:
                    # merged RS: [dk^T et 3-5 (384 rows); dg_hop (768 rows)]
                    m_in = drp.tile([2, EW // 2 + D, NL], BF16, tag="m_in")
                    m_out = drp.tile([EW // 2 + D, NL], BF16, tag="m_out")
                    for r in range(2):
                        nc.sync.dma_start(
                            out=m_in[r, 0 : EW // 2, :].rearrange("(et p) n -> p et n", p=P),
                            in_=dkTst[:, 3:6, r * NL : (r + 1) * NL],
                        )
                        nc.sync.dma_start(
                            out=m_in[r, EW // 2 :, :].rearrange("(dt p) n -> p dt n", p=P),
                            in_=dgh_s[:, :, r * NL : (r + 1) * NL],
                        )
                    nc.gpsimd.collective_compute(
                        "ReduceScatter", OP.add, replica_groups=REPLICA_GROUPS,
                        ins=[m_in.opt()], outs=[m_out.opt()],
                    )
                    nc.sync.dma_start(
                        out=dkT_own[:, 3:6, :],
                        in_=m_out[0 : EW // 2, :].rearrange("(et p) n -> p et n", p=P),
                    )
                    nc.sync.dma_start(
                        out=hopT_own[:], in_=m_out[EW // 2 :, :].rearrange("(dt p) n -> p dt n", p=P),
                    )
                else:
                    nc.vector.tensor_copy(out=hopT_own[:], in_=dgh_s[:, :, 0:NL])
                if debug_dump and step == 0:
                    dump("hopT_own", hopT_own[:])
                    dump("dkT_own", dkT_own[:])

                # ======== attention dgT accumulation (3 pacc banks) ========
                dgTb = [pacc.tile([P, N], F32, tag=f"c{b}", name=f"dga{b}") for b in range(3)]
                for b in range(3):
                    nc.tensor.matmul(dgTb[b][:], zl_t[:], zr_t[:], start=True, stop=False)
                for dt in range(DT):
                    b, half = dt // 2, dt % 2
                    for et in range(ET):
                        nc.tensor.matmul(
                            dgTb[b][:, half * NL : (half + 1) * NL],
                            wqt_sb[:, et, dt * P : (dt + 1) * P], dqTst[:, et, :],
                            start=False, stop=False,
                        )
                for eh in range(2):
                    for dt in range(DT):
                        b, half = dt // 2, dt % 2
                        for et in range(3 * eh, 3 * eh + 3):
                            nc.tensor.matmul(
                                dgTb[b][:, half * NL : (half + 1) * NL],
                                wkt_sb[:, et, dt * P : (dt + 1) * P], dkT_own[:, et, :],
                                start=False, stop=(eh == 1 and et == ET - 1 and half == 1),
                            )

                # ======== combine + transpose + LayerNorm backward ========
                dgTs = work.tile([P, DT, NL], BF16, tag="qT")
                for b in range(3):
                    nc.vector.tensor_tensor(
                        out=dgTs[:, 2 * b : 2 * b + 2, :].rearrange("p t n -> p (t n)"),
                        in0=dgTb[b][:],
                        in1=hopT_own[:, 2 * b : 2 * b + 2, :].rearrange("p t n -> p (t n)"),
                        op=OP.add,
                    )
                if debug_dump and step == 0:
                    dump("dgTs", dgTs[:])
                dg_own = work.tile([P, OC, D], F32, tag="dg_own")
                dxb = work.tile([P, OC, D], BF16, tag="ocd_b")
                m1s = stats.tile([P, OC], F32, tag="m1s")
                u2s = stats.tile([P, OC], F32, tag="u2s")
                for oc in range(OC):
                    pg = ps2.tile([P, D], BF16, tag="ps2")
                    for dt in range(DT):
                        nc.tensor.transpose(pg[:, dt * P : (dt + 1) * P], dgTs[:, dt, oc * P : (oc + 1) * P], ident_b[:])
                    nc.vector.scalar_tensor_tensor(
                        out=dg_own[:, oc, :], in0=pg[:], scalar=rstd_own[:, oc : oc + 1],
                        in1=xhat_own[:, oc, :], op0=OP.mult, op1=OP.bypass,
                        accum_out=m1s[:, oc : oc + 1],
                    )
                    prod = work.tile([P, D], F32, tag="prod")
                    nc.vector.scalar_tensor_tensor(
                        out=prod[:], in0=dg_own[:, oc, :], scalar=1.0, in1=xhat_own[:, oc, :],
                        op0=OP.mult, op1=OP.mult, accum_out=u2s[:, oc : oc + 1],
                    )
                s01 = stats.tile([P, OC, 2], F32, tag="s01")
                nc.vector.tensor_scalar(
                    out=s01[:, :, 0], in0=u2s[:], scalar1=1.0 / D, scalar2=None, op0=OP.mult,
                )
                nc.vector.tensor_scalar(
                    out=s01[:, :, 1], in0=m1s[:], scalar1=1.0 / D, scalar2=None, op0=OP.mult,
                )
                for oc in range(OC):
                    nc.vector.ln_bwd_dx(
                        out=dxb[:, oc, :], dy=dg_own[:, oc, :], x_hat=xhat_own[:, oc, :],
                        mean_dyx=s01[:, oc, 0:1], mean_dy=s01[:, oc, 1:2], scale=ALPHA,
                    )
                if debug_dump and step == 0:
                    dump("dg_own", dg_own[:])
                    dump("s01", s01[:])
                    dump("dxb", dxb[:])

                # ======== pair AllGather of dx; update x (gpsimd) ========
                dxg = work.tile([P, NT, D], BF16, tag="k")
                if with_cc:
                    ag_in = drp.tile([NL, D], BF16, tag="ag_in")
                    ag_out = drp.tile([N, D], BF16, tag="ag_out")
                    nc.sync.dma_start(out=ag_in.rearrange("(oc p) d -> p oc d", p=P), in_=dxb[:])
                    nc.gpsimd.collective_compute(
                        "AllGather", OP.bypass, replica_groups=REPLICA_GROUPS,
                        ins=[ag_in.opt()], outs=[ag_out.opt()],
                    )
                    for nt in range(NT):
                        nc.sync.dma_start(out=dxg[:, nt, :], in_=ag_out[nt * P : (nt + 1) * P, :])
                        nc.vector.scalar_tensor_tensor(
                            out=x_sb[:, nt, :], in0=dxg[:, nt, :], scalar=1.0, in1=x_sb[:, nt, :],
                            op0=OP.mult, op1=OP.add,
                        )
                else:
                    nc.vector.memset(dxg[:], 0.0)
                    nc.vector.tensor_copy(
                        out=dxg[:, 0:OC, :].rearrange("p t d -> p (t d)"),
                        in_=dxb[:].rearrange("p t d -> p (t d)"),
                    )
                    for nt in range(NT):
                        nc.vector.scalar_tensor_tensor(
                            out=x_sb[:, nt, :], in0=dxg[:, nt, :], scalar=1.0, in1=x_sb[:, nt, :],
                            op0=OP.mult, op1=OP.add,
                        )

            for nt in range(NT):
                nc.sync.dma_start(out=x_out[nt * P : (nt + 1) * P, :], in_=x_sb[:, nt, :])

    nc.compile()
    return nc


def _prep_inputs(x, gamma, delta, Wq, Wk, xi):
    """Build the 8 per-core input dicts (host-side sharding + weight folding)."""
    assert np.allclose(delta, 0.0), "kernel requires delta == 0"
    import ml_dtypes

    bf = ml_dtypes.bfloat16
    beta_sqrt = np.float32(1.0 / np.sqrt(np.sqrt(np.float32(HD))))
    g = gamma.astype(np.float32)
    wq = ((Wq * g[None, :, None]).transpose(1, 0, 2).reshape(D, EW) * beta_sqrt).astype(bf)
    wk = ((Wk * g[None, :, None]).transpose(1, 0, 2).reshape(D, EW) * beta_sqrt).astype(bf)
    wqt = ((Wq * g[None, :, None]).transpose(0, 2, 1).reshape(EW, D) / beta_sqrt).astype(bf)
    wkt = ((Wk * g[None, :, None]).transpose(0, 2, 1).reshape(EW, D) / beta_sqrt).astype(bf)
    xi_f = (xi * g[None, :]).astype(np.float32)
    sels, xis, xits = [], [], []
    for j in range(2):
        s = np.zeros((N, NL), dtype=bf)
        s[np.arange(j * NL, (j + 1) * NL), np.arange(NL)] = 1
        sels.append(s)
        sh = xi_f[j * ML : (j + 1) * ML]
        xis.append(np.ascontiguousarray(sh).astype(bf))
        xits.append(np.ascontiguousarray(sh.T).astype(bf))
    in_maps = []
    for c in range(8):
        b, j = c // 2, c % 2
        in_maps.append(
            {
                "x": np.ascontiguousarray(x[b]),
                "sel": sels[j],
                "wq": wq, "wk": wk, "wqt": wqt, "wkt": wkt,
                "xi": xis[j], "xit": xits[j],
            }
        )
    return in_maps


_NC_CACHE = {}


def _get_nc(steps=STEPS, with_cc=True):
    key = (steps, with_cc)
    if key not in _NC_CACHE:
        _NC_CACHE[key] = build_kernel(steps, with_cc)
    return _NC_CACHE[key]


def kernel(x, gamma, delta, Wq, Wk, xi):
    from concourse.bass_utils import run_bass_kernel_spmd

    x = np.asarray(x, dtype=np.float32)
    in_maps = _prep_inputs(
        x,
        np.asarray(gamma, np.float32),
        np.asarray(delta, np.float32),
        np.asarray(Wq, np.float32),
        np.asarray(Wk, np.float32),
        np.asarray(xi, np.float32),
    )
    nc = _get_nc()
    res = run_bass_kernel_spmd(nc, in_maps, list(range(8)))
    out = np.stack([res.results[2 * b]["x_out"] for b in range(B)], axis=0)
    return out.astype(np.float32)


# revision 22
# speedup vs baseline: 1.3619x; 1.1510x over previous
"""Energy Transformer descent kernel for 8 Trainium2 NeuronCores.

Problem: 12 steps of gradient descent on
  E(x) = -(1/beta) sum logsumexp(beta q k^T) - 0.5 sum relu(g xi^T)^2,
  g = LayerNorm(x; gamma, delta), q = g Wq_h, k = g Wk_h.

Sharding: data-parallel over batch B=4 -> core pairs (2b, 2b+1); within a
pair, core j takes attention heads j*6..j*6+5 and Hopfield memories
xi[j*1536:(j+1)*1536].  Both energy terms contribute additively to dE/dx
and LayerNorm-backward is linear in the upstream gradient, so each core
computes a partial dx and a pairwise AllReduce produces the full step.

Host-side preprocessing folds gamma and the attention scale into the
weights (delta must be zero, which the problem guarantees):
  Wq' = sqrt(beta) diag(gamma) Wq      (forward projections)
  WqT' = (1/sqrt(beta)) (diag(gamma) Wq)^T   (gradient projections)
  xi' = xi diag(gamma)
so the kernel never touches gamma/delta and computes true gradients.

Matmul datapath runs in bf16 (weights quantized host-side; fp32 PSUM
accumulation); the dg accumulation, LayerNorm math and the dx exchange
keep fp32/f32r precision where it matters.  dg is accumulated transposed
([d-chunk, n]) so each accumulation chain owns a full PSUM bank.
"""

import numpy as np

import concourse.bass as bass
import concourse.tile as tile
from concourse import bacc, mybir

STEPS = 12
ALPHA = 0.125
EPS = 1e-5
B, N, D, H, HD, M = 4, 512, 768, 12, 64, 3072
P = 128
NT = N // P  # 4 row chunks
DT = D // P  # 6 embed chunks
HL = H // 2  # heads per core
EW = HL * HD  # 384 local head width
ET = EW // P  # 3 stacked head-pair chunks
ML = M // 2  # memories per core
MT = ML // P  # 12 memory chunks
F32 = mybir.dt.float32
F32R = mybir.dt.float32r
BF16 = mybir.dt.bfloat16
AF = mybir.ActivationFunctionType
OP = mybir.AluOpType

REPLICA_GROUPS = [[0, 1], [2, 3], [4, 5], [6, 7]]


def f_(ap):
    return ap.bitcast(F32)




def build_kernel(steps=STEPS, with_ar=True, debug_phase=99, debug_dump=False):
    nc = bacc.Bacc("TRN2", target_bir_lowering=False, debug=False, num_devices=8)

    x_in = nc.declare_dram_parameter("x", [N, D], F32, isOutput=False)
    wq_d = nc.declare_dram_parameter("wq", [D, EW], BF16, isOutput=False)
    wk_d = nc.declare_dram_parameter("wk", [D, EW], BF16, isOutput=False)
    wqt_d = nc.declare_dram_parameter("wqt", [EW, D], BF16, isOutput=False)
    wkt_d = nc.declare_dram_parameter("wkt", [EW, D], BF16, isOutput=False)
    xi_d = nc.declare_dram_parameter("xi", [ML, D], BF16, isOutput=False)
    xit_d = nc.declare_dram_parameter("xit", [D, ML], BF16, isOutput=False)
    x_out = nc.declare_dram_parameter("x_out", [N, D], F32, isOutput=True)
    dbg = {}
    if debug_dump:
        for nm, shp in (("xhat", [N, D]), ("gT", [D, N]), ("q", [N, EW]), ("kT", [EW, N]),
                        ("P0", [N, N]), ("dqT", [EW, N]), ("dg", [N, D]), ("dx", [N, D])):
            dbg[nm] = nc.declare_dram_parameter("o_" + nm, shp, F32, isOutput=True)

    with tile.TileContext(nc) as tc:
        import contextlib

        with contextlib.ExitStack() as ctx:
            consts = ctx.enter_context(tc.tile_pool(name="consts", bufs=1))
            work = ctx.enter_context(tc.tile_pool(name="work", bufs=1))
            attp = ctx.enter_context(tc.tile_pool(name="attp", bufs=2))
            stats = ctx.enter_context(tc.tile_pool(name="stats", bufs=4))
            stream = ctx.enter_context(tc.tile_pool(name="stream", bufs=3))
            rtp = ctx.enter_context(tc.tile_pool(name="rtp", bufs=3))
            scr = ctx.enter_context(tc.tile_pool(name="scr", bufs=2))
            ps = ctx.enter_context(tc.tile_pool(name="ps", bufs=2, space="PSUM"))
            drp = ctx.enter_context(tc.tile_pool(name="drp", bufs=2, space="DRAM"))

            # ---- resident tensors ----
            wq_sb = consts.tile([P, DT, EW], BF16)
            nc.sync.dma_start(out=wq_sb[:], in_=wq_d.rearrange("(dt p) e -> p dt e", p=P))
            wk_sb = consts.tile([P, DT, EW], BF16)
            nc.sync.dma_start(out=wk_sb[:], in_=wk_d.rearrange("(dt p) e -> p dt e", p=P))
            wqt_sb = consts.tile([P, ET, D], BF16)
            nc.sync.dma_start(out=wqt_sb[:], in_=wqt_d.rearrange("(et p) d -> p et d", p=P))
            wkt_sb = consts.tile([P, ET, D], BF16)
            nc.sync.dma_start(out=wkt_sb[:], in_=wkt_d.rearrange("(et p) d -> p et d", p=P))
            x_sb = consts.tile([P, NT, D], F32)
            nc.sync.dma_start(out=x_sb[:], in_=x_in.rearrange("(nt p) d -> p nt d", p=P))

            from concourse.masks import make_identity

            ident_f = consts.tile([P, P], F32)
            make_identity(nc, ident_f[:])
            ident = consts.tile([P, P], F32R)
            nc.vector.tensor_copy(out=ident[:], in_=ident_f[:])
            ident_b = consts.tile([P, P], BF16)
            nc.vector.tensor_copy(out=ident_b[:], in_=ident_f[:])
            eps_t = consts.tile([P, 1], F32)
            nc.vector.memset(eps_t[:], EPS)

            for step in range(steps):
                # ======== LayerNorm forward ========
                xhat = work.tile([P, NT, D], BF16, tag="xhat")
                rstd = stats.tile([P, NT], F32, tag="rstd")
                for nt in range(NT):
                    xt = x_sb[:, nt, :]
                    st = stats.tile([P, 3, 6], F32, tag="bnst")
                    xg = xt.rearrange("p (g s) -> p g s", s=256)
                    for gs in range(3):
                        nc.vector.bn_stats(out=st[:, gs, :], in_=xg[:, gs, :])
                    mv = stats.tile([P, 2], F32, tag="mv")
                    nc.vector.bn_aggr(out=mv[:], in_=st[:])
                    rr = rstd[:, nt : nt + 1]
                    nc.scalar.activation(out=rr, in_=mv[:, 1:2], func=AF.Sqrt, bias=eps_t[:], scale=1.0)
                    nc.vector.reciprocal(out=rr, in_=rr)
                    nmu = stats.tile([P, 1], F32, tag="nmu")
                    nc.vector.scalar_tensor_tensor(
                        out=nmu[:], in0=mv[:, 0:1], scalar=-1.0, in1=rr, op0=OP.mult, op1=OP.mult,
                    )
                    nc.vector.tensor_scalar(
                        out=xhat[:, nt, :], in0=xt, scalar1=rr, scalar2=nmu[:],
                        op0=OP.mult, op1=OP.add,
                    )

                if debug_phase < 2:
                    continue
                # gT = xhat^T  [d-part, n-free]
                psw_ctx = tc.tile_pool(name="psw", bufs=6, space="PSUM")
                psw = psw_ctx.__enter__()
                gT = work.tile([P, DT, N], BF16, tag="gT")
                for dt in range(DT):
                    pt = psw.tile([P, 512], BF16, tag="psw")
                    for nt in range(NT):
                        nc.tensor.transpose(pt[:, nt * P : (nt + 1) * P], xhat[:, nt, dt * P : (dt + 1) * P], ident_b[:])
                    nc.vector.tensor_copy(out=gT[:, dt, :], in_=pt[:])

                if debug_dump and step == 0:
                    nc.sync.dma_start(out=dbg["xhat"].rearrange("(nt p) d -> p nt d", p=P), in_=f_(xhat[:]))
                    nc.sync.dma_start(out=dbg["gT"].rearrange("(dt p) n -> p dt n", p=P), in_=f_(gT[:]))
                if debug_phase < 3:
                    continue
                # ======== projections ========
                q = work.tile([P, NT, EW], BF16, tag="q")
                k = work.tile([P, NT, EW], BF16, tag="k")
                for nt in range(NT):
                    ppq = psw.tile([P, 512], F32, tag="psw")
                    ppk = psw.tile([P, 512], F32, tag="psw")
                    for dt in range(DT):
                        lh = gT[:, dt, nt * P : (nt + 1) * P]
                        nc.tensor.matmul(ppq[:, :EW], lh, wq_sb[:, dt, :], start=(dt == 0), stop=(dt == DT - 1))
                        nc.tensor.matmul(ppk[:, :EW], lh, wk_sb[:, dt, :], start=(dt == 0), stop=(dt == DT - 1))
                    nc.vector.tensor_copy(out=q[:, nt, :], in_=ppq[:, :EW])
                    nc.vector.tensor_copy(out=k[:, nt, :], in_=ppk[:, :EW])
                qT = work.tile([P, ET, N], BF16, tag="qT")
                kT = work.tile([P, ET, N], BF16, tag="kT")
                for dst, srct in ((qT, q), (kT, k)):
                    for et in range(ET):
                        pp = psw.tile([P, 512], BF16, tag="psw")
                        for nt in range(NT):
                            nc.tensor.transpose(
                                pp[:, nt * P : (nt + 1) * P],
                                srct[:, nt, et * P : (et + 1) * P], ident_b[:],
                            )
                        nc.vector.tensor_copy(out=dst[:, et, :], in_=pp[:])

                if debug_dump and step == 0:
                    nc.sync.dma_start(out=dbg["q"].rearrange("(nt p) e -> p nt e", p=P), in_=f_(q[:]))
                    nc.sync.dma_start(out=dbg["kT"].rearrange("(et p) n -> p et n", p=P), in_=f_(kT[:]))
                if debug_phase < 4:
                    continue
                # ======== attention heads ========
                dqTst = work.tile([P, ET, N], BF16, tag="dqTst")
                dkTst = work.tile([P, ET, N], BF16, tag="dkTst")
                for h in range(HL):
                    et, eo = h // 2, (h % 2) * HD
                    Pn = attp.tile([P, NT, N], BF16, tag="Pn")
                    PTn = attp.tile([P, NT, N], BF16, tag="PTn")
                    for nt in range(NT):
                        sc = psw.tile([P, 512], F32, tag="psw")
                        nc.tensor.matmul(
                            sc[:], qT[eo : eo + HD, et, nt * P : (nt + 1) * P],
                            kT[eo : eo + HD, et, :], start=True, stop=True,
                        )
                        sm = stats.tile([P, 1], F32, tag="sm")
                        nc.scalar.activation(
                            out=Pn[:, nt, :], in_=sc[:], func=AF.Exp, bias=0.0, scale=1.0,
                            accum_out=sm[:],
                        )
                        nc.vector.reciprocal(out=sm[:], in_=sm[:])
                        nc.vector.tensor_scalar_mul(out=Pn[:, nt, :], in0=Pn[:, nt, :], scalar1=sm[:])
                    if debug_dump and step == 0 and h == 0:
                        nc.sync.dma_start(out=dbg["P0"].rearrange("(nt p) m -> p nt m", p=P), in_=f_(Pn[:]))
                    # PT via PE transposes (4 transposes share one psum tile)
                    for mt in range(NT):
                        pt = psw.tile([P, 512], BF16, tag="psw")
                        for nt in range(NT):
                            nc.tensor.transpose(pt[:, nt * P : (nt + 1) * P], Pn[:, nt, mt * P : (mt + 1) * P], ident_b[:])
                        nc.vector.tensor_copy(out=PTn[:, mt, :], in_=pt[:])
                    # dqT_h = sum_mt k_h[mt]^T-as-lhsT @ PT[mt]
                    pp = psw.tile([P, 512], F32, tag="psw")
                    for mt in range(NT):
                        nc.tensor.matmul(
                            pp[:HD, :], k[:, mt, h * HD : (h + 1) * HD], PTn[:, mt, :],
                            start=(mt == 0), stop=(mt == NT - 1),
                        )
                    nc.vector.tensor_copy(out=dqTst[eo : eo + HD, et, :], in_=pp[:HD, :])
                    # dkT_h = sum_nt q_h[nt]-as-lhsT @ P[nt]
                    pp2 = psw.tile([P, 512], F32, tag="psw")
                    for nt in range(NT):
                        nc.tensor.matmul(
                            pp2[:HD, :], q[:, nt, h * HD : (h + 1) * HD], Pn[:, nt, :],
                            start=(nt == 0), stop=(nt == NT - 1),
                        )
                    nc.vector.tensor_copy(out=dkTst[eo : eo + HD, et, :], in_=pp2[:HD, :])

                if debug_dump and step == 0:
                    nc.sync.dma_start(out=dbg["dqT"].rearrange("(et p) n -> p et n", p=P), in_=f_(dqTst[:]))
                psw_ctx.__exit__(None, None, None)
                if debug_phase < 5:
                    continue
                # ======== dg accumulation in PSUM, transposed [d-chunk, n] ========
                # dgT (= -true dg^T): each d-chunk owns a full PSUM bank so every
                # accumulation chain is bank-exclusive (PSUM has_written clears are
                # bank-wide; two chains must never share a bank).
                psdg_ctx = tc.tile_pool(name="psdg", bufs=1, space="PSUM")
                psdg = psdg_ctx.__enter__()
                dgTb = [psdg.tile([P, N], F32, tag=f"dgT{dt}", name=f"dgT{dt}") for dt in range(DT)]
                for dt in range(DT):
                    first = True
                    for et in range(ET):
                        for d_t, w_t in ((dqTst, wqt_sb), (dkTst, wkt_sb)):
                            nc.tensor.matmul(
                                dgTb[dt][:], w_t[:, et, dt * P : (dt + 1) * P],
                                d_t[:, et, :], start=first, stop=False,
                            )
                            first = False

                # ======== hopfield ========
                for mt in range(MT):
                    xitm = stream.tile([P, DT, P], BF16, tag="xitm")
                    nc.sync.dma_start(
                        out=xitm[:],
                        in_=xit_d[:, mt * P : (mt + 1) * P].rearrange("(dt p) m -> p dt m", p=P),
                    )
                    hp = ps.tile([P, 512], F32, tag="ps")
                    for dt in range(DT):
                        nc.tensor.matmul(
                            hp[:], xitm[:, dt, :], gT[:, dt, :],
                            start=(dt == 0), stop=(dt == DT - 1),
                        )
                    RT = rtp.tile([P, N], BF16, tag="RT")
                    nc.scalar.activation(out=RT[:], in_=hp[:], func=AF.Relu)
                    xim = stream.tile([P, D], BF16, tag="xim")
                    nc.sync.dma_start(out=xim[:], in_=xi_d[mt * P : (mt + 1) * P, :])
                    last = mt == MT - 1
                    for dt in range(DT):
                        nc.tensor.matmul(
                            dgTb[dt][:], xim[:, dt * P : (dt + 1) * P], RT[:],
                            start=False, stop=last,
                        )

                if debug_phase < 7:
                    continue
                # ======== transpose dg back to [n-part, d]; fused LN backward ========
                dgTs = work.tile([P, DT, N], BF16, tag="dgTs")
                for dt in range(DT):
                    nc.vector.tensor_copy(out=dgTs[:, dt, :], in_=dgTb[dt][:])
                psdg_ctx.__exit__(None, None, None)
                dx = work.tile([P, NT, D], F32, tag="dx")
                dxb = work.tile([P, NT, D], BF16, tag="dxb")
                m1s = stats.tile([P, 2, NT], F32, tag="m1s")
                u2s = stats.tile([P, NT], F32, tag="u2s")
                for nt in range(NT):
                    rr = rstd[:, nt : nt + 1]
                    pt = ps.tile([P, 512], BF16, tag="ps")
                    for dt in range(4):
                        nc.tensor.transpose(pt[:, dt * P : (dt + 1) * P], dgTs[:, dt, nt * P : (nt + 1) * P], ident_b[:])
                    nc.vector.scalar_tensor_tensor(
                        out=dx[:, nt, 0:512], in0=pt[:], scalar=rr, in1=xhat[:, nt, 0:512],
                        op0=OP.mult, op1=OP.bypass, accum_out=m1s[:, 0, nt : nt + 1],
                    )
                    pt2 = ps.tile([P, 512], BF16, tag="ps")
                    for dt in range(4, DT):
                        nc.tensor.transpose(pt2[:, (dt - 4) * P : (dt - 3) * P], dgTs[:, dt, nt * P : (nt + 1) * P], ident_b[:])
                    nc.vector.scalar_tensor_tensor(
                        out=dx[:, nt, 512:768], in0=pt2[:, :256], scalar=rr, in1=xhat[:, nt, 512:768],
                        op0=OP.mult, op1=OP.bypass, accum_out=m1s[:, 1, nt : nt + 1],
                    )
                    prodA = scr.tile([P, D], F32, tag="prodA")
                    nc.vector.scalar_tensor_tensor(
                        out=prodA[:], in0=dx[:, nt, :], scalar=1.0, in1=xhat[:, nt, :],
                        op0=OP.mult, op1=OP.mult, accum_out=u2s[:, nt : nt + 1],
                    )
                s01 = stats.tile([P, NT, 2], F32, tag="s01")
                for nt in range(NT):
                    nc.vector.tensor_tensor(
                        out=s01[:, nt, 1:2], in0=m1s[:, 0, nt : nt + 1], in1=m1s[:, 1, nt : nt + 1], op=OP.add,
                    )
                nc.vector.tensor_scalar(
                    out=s01[:, :, 1], in0=s01[:, :, 1], scalar1=1.0 / D, scalar2=None, op0=OP.mult,
                )
                nc.vector.tensor_scalar(
                    out=s01[:, :, 0], in0=u2s[:], scalar1=1.0 / D, scalar2=None, op0=OP.mult,
                )
                for nt in range(NT):
                    nc.vector.ln_bwd_dx(
                        out=dxb[:, nt, :], dy=dx[:, nt, :], x_hat=xhat[:, nt, :],
                        mean_dyx=s01[:, nt, 0:1], mean_dy=s01[:, nt, 1:2], scale=ALPHA,
                    )

                if debug_dump and step == 0:
                    nc.sync.dma_start(out=dbg["dx"].rearrange("(nt p) d -> p nt d", p=P), in_=dx[:])
                # ======== pair AllReduce + update ========
                if with_ar:
                    arin = drp.tile([N, D], BF16, tag="arin")
                    arout = drp.tile([N, D], BF16, tag="arout")
                    for nt in range(NT):
                        nc.sync.dma_start(out=arin[nt * P : (nt + 1) * P, :], in_=dxb[:, nt, :])
                    nc.gpsimd.collective_compute(
                        "AllReduce", OP.add, replica_groups=REPLICA_GROUPS,
                        ins=[arin.opt()], outs=[arout.opt()],
                    )
                    nc.sync.dma_start(out=dxb[:], in_=arout.rearrange("(nt p) d -> p nt d", p=P))
                if debug_phase < 12:
                    continue
                for nt in range(NT):
                    nc.vector.scalar_tensor_tensor(
                        out=x_sb[:, nt, :], in0=dxb[:, nt, :], scalar=1.0, in1=x_sb[:, nt, :],
                        op0=OP.mult, op1=OP.add,
                    )

            for nt in range(NT):
                nc.sync.dma_start(out=x_out[nt * P : (nt + 1) * P, :], in_=x_sb[:, nt, :])

    nc.compile()
    return nc


def _prep_inputs(x, gamma, delta, Wq, Wk, xi):
    """Build the 8 per-core input dicts (host-side sharding + weight folding)."""
    assert np.allclose(delta, 0.0), "kernel requires delta == 0"
    beta_sqrt = np.float32(1.0 / np.sqrt(np.sqrt(np.float32(HD))))
    # sqrt(beta) = (1/sqrt(HD))^(1/2) = HD^(-1/4)
    g = gamma.astype(np.float32)
    in_maps = []
    for c in range(8):
        b, j = c // 2, c % 2
        hs = slice(j * HL, (j + 1) * HL)
        wq_l = (Wq[hs] * g[None, :, None]).transpose(1, 0, 2).reshape(D, EW)
        wk_l = (Wk[hs] * g[None, :, None]).transpose(1, 0, 2).reshape(D, EW)
        wqt_l = (Wq[hs] * g[None, :, None]).transpose(0, 2, 1).reshape(EW, D)
        wkt_l = (Wk[hs] * g[None, :, None]).transpose(0, 2, 1).reshape(EW, D)
        xi_l = xi[j * ML : (j + 1) * ML] * g[None, :]
        import ml_dtypes

        bf = ml_dtypes.bfloat16
        in_maps.append(
            {
                "x": np.ascontiguousarray(x[b]),
                "wq": np.ascontiguousarray(wq_l * beta_sqrt).astype(bf),
                "wk": np.ascontiguousarray(wk_l * beta_sqrt).astype(bf),
                "wqt": np.ascontiguousarray(wqt_l / beta_sqrt).astype(bf),
                "wkt": np.ascontiguousarray(wkt_l / beta_sqrt).astype(bf),
                "xi": np.ascontiguousarray(xi_l).astype(bf),
                "xit": np.ascontiguousarray(xi_l.T).astype(bf),
            }
        )
    return in_maps


_NC_CACHE = {}


def _get_nc(steps=STEPS, with_ar=True):
    key = (steps, with_ar)
    if key not in _NC_CACHE:
        _NC_CACHE[key] = build_kernel(steps, with_ar)
    return _NC_CACHE[key]


def kernel(x, gamma, delta, Wq, Wk, xi):
    from concourse.bass_utils import run_bass_kernel_spmd

    x = np.asarray(x, dtype=np.float32)
    in_maps = _prep_inputs(
        x,
        np.asarray(gamma, np.float32),
        np.asarray(delta, np.float32),
        np.asarray(Wq, np.float32),
        np.asarray(Wk, np.float32),
        np.asarray(xi, np.float32),
    )
    nc = _get_nc()
    res = run_bass_kernel_spmd(nc, in_maps, list(range(8)))
    out = np.stack([res.results[2 * b]["x_out"] for b in range(B)], axis=0)
    return out.astype(np.float32)

